# revision 4
# baseline (speedup 1.0000x reference)
"""Trainium2 Bass kernel for causal masked-ReLU attention (no softmax).

Reference computation (B=8, T=1024, C=768, n_head=12, hd=64):
    qkv = x @ W_attn.T + b_attn
    q, k, v = split(qkv); per-head: att = relu(mask_causal(q k^T / sqrt(hd)))
    y = att @ v, heads re-merged -> (B, T, C)

Sharding: one batch element per NeuronCore (8 cores). Each core computes the
QKV projection and all 12 heads' attention for its batch element.

Layout strategy (per core):
  - Host passes x[b].T (C, T) and W.T (C, 3C) so the contraction dim C lands
    on SBUF partitions with unit-stride DMA (no on-chip transposes).
  - W rows are pre-permuted on host into [q-pair0, k-pair0, q-pair1, ...] so
    q.T / k.T of head h live at the same partition offset (h%2)*64 of their
    M-tiles; q weights/bias pre-scaled by 1/sqrt(hd).
  - QKV projection runs in fp8 (e4m3) DoubleRow perf mode: 256-deep
    contraction per pass at 0.5 cycles/row. Operands split into hi+lo fp8
    digits and three digit products xh*wh + xl*wh + xh*wl accumulate in one
    fp32 PSUM group (dropped xl*wl ~1e-4 relative); the 2^13 operand scale
    comes out at eviction with the bias. PE cost 0.75x of fp16.
  - att is computed transposed (att.T = k @ q.T, layout [T_k, T_q]), fp16
    everywhere, BOTH heads of a pair sharing one [128, 2, 512] double-bank
    PSUM tile and one [128, 2, T] SBUF tile per k-tile: the per-instruction
    PSUM-access penalty on DVE/ACT (~125/185ns) dominates phase 2, and
    pair-merging halves the eviction op count.
  - att tiles are double-buffered ACROSS pairs (sets alternate a%2): pair
    a+1's QK pieces stream through DVE/ACT while pair a's AV still reads
    the other set, so the mask-bound DVE runs and relu-bound ACT runs
    interleave instead of convoying (Pool cannot read PSUM, so evictions
    have only these two engines).
  - Per-pair piece order tk4..tk7, then [512:T) halves of tk0-3, then
    diagonal halves of tk0-3: AV bank-high's dependencies resolve
    mid-pair and bank-low's at the end, so the closing chain after the
    final AV matmul is one [128, 256] copy + one small DMA.
  - The AV product exploits weight-stationary asymmetry: per (q-tile,
    k-tile, head) matmul the STATIONARY operand is the [128, 128] att.T
    block and the MOVING operand is the head's 64 v columns (Ldweights is
    free; cost follows moving columns). Both heads accumulate into one
    [128, 512] PSUM bank (cols = 128*(t%4) + 64*head_parity + d); y
    DMAs out with a 3-d strided AP into natural (T, C) layout.
  - Causal masking at eviction via a [tri(128) | ones] relu-mask; DVE owns
    mask-needing pieces (only PSUM-capable engine with
    scalar_tensor_tensor), ACT the pure-relu halves; tile 4 splits
    [masked 2x128 | relu 2x384] to converge both at ~6.0us/pair, just
    above PE's 5.8us/pair.
  - The input front is split fine (x column halves) and spread across the
    SP HWDGE queue and the Pool SWDGE queue (which bypasses the shared
    HWDGE's ~625ns/DMA): the first window's eviction gates on all three
    contraction pairs, and a single queue starves the PE ~3us.
  - Warmup matmuls on a zeroed scratch tile ramp the PE p-state during the
    initial DMA wait, with more sprinkled into the first windows' stall
    points (idle gaps reset the ~3us ramp).
  - Projection windows: 2 groups per double-bank tile, 3 tiles rotating;
    mid-stream windows borrow the two spare ps_y banks for 8 groups in
    flight. One pool scope spans both phases (pool close = all-engine
    barrier); pair 0's first two k-tiles run unmerged through ps_y so
    phase 2 overlaps the last projection windows.
  - Output is written as y (T, C) in fp16; host upcasts.
"""

import numpy as np

import sys
for _p in ("/opt/trn_rl_repo", "/root/.axon_site", "/root/.axon_site/_ro/trn_rl_repo",
           "/root/.axon_site/_ro/pypackages"):
    if _p not in sys.path:
        sys.path.append(_p)

import ml_dtypes

import concourse.bacc as bacc
import concourse.mybir as mybir
from concourse.alu_op_type import AluOpType
from concourse.tile import TileContext
from concourse.bass_utils import run_bass_kernel_spmd

B, T, C = 8, 1024, 768
NH, HD = 12, 64
C3 = 3 * C            # 2304
KT = C // 128         # 6  contraction tiles of the projection
NP = KT // 2          # 3  contraction pairs (DoubleRow)
TT = T // 128         # 8  tiles of the sequence dim
NPAIR = NH // 2       # 6  head pairs
F32 = mybir.dt.float32
F16 = mybir.dt.float16
F8 = mybir.dt.float8e4
AF = mybir.ActivationFunctionType
DR = mybir.MatmulPerfMode.DoubleRow

SX = 16.0             # x pre-scale (keeps x-lo digits in e4m3 normal range)
SW = 512.0            # W pre-scale
DESCALE = 1.0 / (SX * SW)

# warmup matmul moving-widths (fp16): ramp the PE p-state during the
# initial input-DMA wait so real matmuls start at full clock
WARM = [512] * 4 + [256]
NSPRINKLE = 8         # 256-wide ramp-guard warmups per early stall point

_CACHE = {}


def _build():
    nc = bacc.Bacc("TRN2", target_bir_lowering=False, debug=False, num_devices=8)

    xh = nc.dram_tensor("xh", [128, KT, T], F8, kind="ExternalInput").ap()
    xl = nc.dram_tensor("xl", [128, KT, T], F8, kind="ExternalInput").ap()
    wvh = nc.dram_tensor("wvh", [128, KT, C], F8, kind="ExternalInput").ap()
    wvl = nc.dram_tensor("wvl", [128, KT, C], F8, kind="ExternalInput").ap()
    wqh = nc.dram_tensor("wqh", [128, KT, 2 * C], F8, kind="ExternalInput").ap()
    wql = nc.dram_tensor("wql", [128, KT, 2 * C], F8, kind="ExternalInput").ap()
    bqk = nc.dram_tensor("bqk", [128, 2 * NPAIR], F32, kind="ExternalInput").ap()
    bvb = nc.dram_tensor("bvb", [128, C], F16, kind="ExternalInput").ap()
    # masks = [tri(128) | ones(896)] duplicated along dim1 so pair-merged
    # [128, 2, W] evictions read the same relu-mask for both heads
    masks = nc.dram_tensor("masks", [128, 2, T], F32, kind="ExternalInput").ap()
    # y in natural (T, C) layout, tiled (TT, 128, C) for the 3-d AV DMAs
    y_d = nc.dram_tensor("y", [TT, 128, C], F16, kind="ExternalOutput").ap()

    with TileContext(nc) as tc:
        with (
            tc.tile_pool(name="persist", bufs=1) as pp,
        ):
            masks_sb = pp.tile([128, 2, T], F32, name="masks_sb")
            bqk_sb = pp.tile([128, 2 * NPAIR], F32, name="bqk_sb")
            bvb_sb = pp.tile([128, C], F16, name="bvb_sb")
            qkT = [pp.tile([128, T], F16, name=f"qkT{m}") for m in range(2 * NPAIR)]
            v_sb = [pp.tile([128, C], F16, name=f"v{t}") for t in range(TT)]
            # att.T tiles, fp16, dim1 = head parity; two sets alternating
            # per pair so pair a+1's QK overlaps pair a's AV
            att2 = [[pp.tile([128, 2, T], F16, name=f"att{s}_{t}")
                     for t in range(TT)] for s in range(2)]

            # ---------- Phase 1: QKV projection (fp8 DoubleRow, 3 digit
            # products into one PSUM group) ----------
            from contextlib import ExitStack
            with ExitStack() as stack:
                iop = stack.enter_context(tc.tile_pool(name="io", bufs=1))
                # 3 double-bank tiles (12KB/partition) shared by projection
                # windows and phase-2 merged QK pieces; + 2 single banks for
                # warmup / window-borrow / early-QK / AV
                ps_proj = stack.enter_context(
                    tc.tile_pool(name="psum_proj", bufs=3, space="PSUM"))
                ps_y = stack.enter_context(
                    tc.tile_pool(name="psum_y", bufs=2, space="PSUM"))
                yop = stack.enter_context(tc.tile_pool(name="yout", bufs=2))
                xh_sb = iop.tile([128, KT, T], F8, name="xh_sb")
                xl_sb = iop.tile([128, KT, T], F8, name="xl_sb")
                wv_h = iop.tile([128, KT, C], F8, name="wv_h")
                wv_l = iop.tile([128, KT, C], F8, name="wv_l")
                wq_h = iop.tile([128, KT, 2 * C], F8, name="wq_h")
                wq_l = iop.tile([128, KT, 2 * C], F8, name="wq_l")

                # PE p-state warmup on a zeroed scratch tile; results are
                # never read
                scratch = iop.tile([128, 512], F16, name="warm_src")
                nc.vector.memset(scratch[:], 0.0)
                warm = ps_y.tile([128, 512], F32, name="ps_warm", tag="ps_y")
                for w in WARM:
                    nc.tensor.matmul(warm[:, :w], scratch[:, :128],
                                     scratch[:, :w], start=True, stop=True)

                # input DMAs. The first windows' evictions gate on ALL three
                # contraction pairs, so the front ships x in column halves
                # and splits across the SP HWDGE queue and the Pool SWDGE
                # queue (parallel descriptor generators).
                sp, pool = nc.sync, nc.gpsimd
                sp.dma_start(out=wv_h[:, 0:2, :], in_=wvh[:, 0:2, :])
                sp.dma_start(out=xh_sb[:, 0:2, 0:256], in_=xh[:, 0:2, 0:256])
                pool.dma_start(out=xl_sb[:, 0:2, 0:512], in_=xl[:, 0:2, 0:512])
                sp.dma_start(out=wv_l[:, 0:2, :], in_=wvl[:, 0:2, :])
                sp.dma_start(out=xh_sb[:, 0:2, 256:512], in_=xh[:, 0:2, 256:512])
                pool.dma_start(out=xh_sb[:, 2:4, 0:512], in_=xh[:, 2:4, 0:512])
                sp.dma_start(out=wv_h[:, 2:4, :], in_=wvh[:, 2:4, :])
                pool.dma_start(out=xl_sb[:, 2:4, 0:512], in_=xl[:, 2:4, 0:512])
                sp.dma_start(out=wv_l[:, 2:4, :], in_=wvl[:, 2:4, :])
                pool.dma_start(out=xh_sb[:, 4:6, 0:512], in_=xh[:, 4:6, 0:512])
                sp.dma_start(out=wv_h[:, 4:6, :], in_=wvh[:, 4:6, :])
                pool.dma_start(out=xl_sb[:, 4:6, 0:512], in_=xl[:, 4:6, 0:512])
                sp.dma_start(out=wv_l[:, 4:6, :], in_=wvl[:, 4:6, :])
                sp.dma_start(out=bvb_sb[:], in_=bvb[:])
                # x column rests (v tiles 4-7 of the seq dim; windows 2-3)
                for p in range(NP):
                    pr = slice(2 * p, 2 * p + 2)
                    sp.dma_start(out=xh_sb[:, pr, 512:T], in_=xh[:, pr, 512:T])
                    pool.dma_start(out=xl_sb[:, pr, 512:T], in_=xl[:, pr, 512:T])
                # q/k weights; pair 0 in m0-m3 / m4-m11 halves
                pr0 = slice(0, 2)
                sp.dma_start(out=wq_h[:, pr0, :512], in_=wqh[:, pr0, :512])
                pool.dma_start(out=wq_l[:, pr0, :512], in_=wql[:, pr0, :512])
                sp.dma_start(out=wq_h[:, pr0, 512:], in_=wqh[:, pr0, 512:])
                pool.dma_start(out=wq_l[:, pr0, 512:], in_=wql[:, pr0, 512:])
                sp.dma_start(out=bqk_sb[:], in_=bqk[:])
                for p in range(1, NP):
                    prp = slice(2 * p, 2 * p + 2)
                    sp.dma_start(out=wq_h[:, prp, :], in_=wqh[:, prp, :])
                    pool.dma_start(out=wq_l[:, prp, :], in_=wql[:, prp, :])
                sp.dma_start(out=masks_sb[:], in_=masks[:])

                # each group = one [128, 512] PSUM bank lane.
                # ("v", t, n0, width) / ("qk", m, q0, width)
                groups = []
                for t in range(TT):
                    groups.append(("v", t, 0, 512))
                    groups.append(("v", t, 512, 256))
                for m in range(2 * NPAIR):
                    for q0 in (0, 512):
                        groups.append(("qk", m, q0, 512))

                # windows of 4 groups; k-pair-major, digit-product-minor so
                # PE consumption order matches DMA arrival order. Mid-stream
                # windows borrow the two ps_y banks for deeper pipelining.
                nwin = (len(groups) + 3) // 4
                for wi, w0 in enumerate(range(0, len(groups), 4)):
                    window = groups[w0:w0 + 4]
                    borrow = 3 <= wi < nwin - 2
                    if borrow:
                        dbl = ps_proj.tile([128, 2, 512], F32,
                                           name="ps_proj", tag="ps_proj")
                        tiles = [dbl[:, 0, :], dbl[:, 1, :],
                                 ps_y.tile([128, 512], F32, name="ps_b",
                                           tag="ps_y"),
                                 ps_y.tile([128, 512], F32, name="ps_b",
                                           tag="ps_y")][:len(window)]
                    else:
                        dbl = [ps_proj.tile([128, 2, 512], F32,
                                            name="ps_proj", tag="ps_proj")
                               for _ in range((len(window) + 1) // 2)]
                        tiles = [dbl[gi // 2][:, gi % 2, :]
                                 for gi in range(len(window))]
                    nmm = [0] * len(window)
                    total = [9 * (g[3] // 256) for g in window]
                    for p in range(NP):
                        pr = slice(2 * p, 2 * p + 2)
                        for term in range(3):
                            xa = (xh_sb, xl_sb, xh_sb)[term]
                            wva = (wv_h, wv_h, wv_l)[term]
                            wqa = (wq_h, wq_h, wq_l)[term]
                            for gi, (g, ps) in enumerate(zip(window, tiles)):
                                kind, i, o0, wd = g
                                for c0 in range(0, wd, 256):
                                    n = nmm[gi]
                                    nmm[gi] = n + 1
                                    st = n == 0
                                    sp_ = n == total[gi] - 1
                                    if kind == "v":
                                        nc.tensor.matmul(
                                            ps[:, c0:c0 + 256],
                                            xa[:, pr, 128 * i:128 * (i + 1)],
                                            wva[:, pr, o0 + c0:o0 + c0 + 256],
                                            start=st, stop=sp_, perf_mode=DR,
                                        )
                                    else:
                                        nc.tensor.matmul(
                                            ps[:, c0:c0 + 256],
                                            wqa[:, pr, 128 * i:128 * (i + 1)],
                                            xa[:, pr, o0 + c0:o0 + c0 + 256],
                                            start=st, stop=sp_, perf_mode=DR,
                                        )
                            if wi < 2 and term == 2:
                                # ramp-guard warmups at the early stall
                                # points (pair p+1 still in flight)
                                for _ in range(NSPRINKLE):
                                    nc.tensor.matmul(
                                        warm[:, :256], scratch[:, :128],
                                        scratch[:, :256], start=True,
                                        stop=True)
                    for g, ps in zip(window, tiles):
                        kind, i, o0, wd = g
                        if kind == "v":
                            nc.vector.scalar_tensor_tensor(
                                v_sb[i][:, o0:o0 + wd], ps[:, :wd], DESCALE,
                                bvb_sb[:, o0:o0 + wd],
                                AluOpType.mult, AluOpType.add,
                            )
                        elif i % 2 == 0:
                            nc.scalar.activation(
                                qkT[i][:, o0:o0 + wd], ps[:, :wd],
                                AF.Identity, bias=bqk_sb[:, i:i + 1],
                                scale=DESCALE,
                            )
                        else:
                            nc.vector.tensor_scalar(
                                qkT[i][:, o0:o0 + wd], ps[:, :wd],
                                DESCALE, bqk_sb[:, i:i + 1],
                                AluOpType.mult, AluOpType.add,
                            )

            # ---------- Phase 2: attention, pair by pair (same pool
            # scope: no phase barrier) ----------
                def qk_piece(att, qa, ka, tk, lo, hi, engine):
                    """One pair-merged QK piece covering q cols [lo, hi);
                    engine: 'dve' masked relu / 'act' pure relu / 'mix'
                    masked first 128 cols on DVE + relu rest on ACT."""
                    k0 = 128 * tk
                    ps = ps_proj.tile([128, 2, 512], F32, name="ps_qk",
                                      tag="ps_proj")
                    for r in range(2):
                        nc.tensor.matmul(
                            ps[:, r, :hi - lo],
                            ka[64 * r:64 * (r + 1), k0:k0 + 128],
                            qa[64 * r:64 * (r + 1), lo:hi],
                            start=True, stop=True,
                        )
                    if engine == "act":
                        nc.scalar.activation(att[tk][:, :, lo:hi],
                                             ps[:, :, :hi - lo], AF.Relu)
                    elif engine == "dve":
                        nc.vector.scalar_tensor_tensor(
                            att[tk][:, :, lo:hi], ps[:, :, :hi - lo],
                            0.0, masks_sb[:, :, :hi - lo],
                            AluOpType.max, AluOpType.mult,
                        )
                    else:  # mix: diag block on DVE, remainder on ACT
                        nc.vector.scalar_tensor_tensor(
                            att[tk][:, :, lo:lo + 128], ps[:, :, 0:128],
                            0.0, masks_sb[:, :, :128],
                            AluOpType.max, AluOpType.mult,
                        )
                        nc.scalar.activation(att[tk][:, :, lo + 128:hi],
                                             ps[:, :, 128:hi - lo], AF.Relu)

                for a in range(NPAIR):
                    att = att2[a % 2]
                    qa, ka = qkT[2 * a], qkT[2 * a + 1]
                    if a == 0:
                        # tk0/tk1 as unmerged singles through the spare ps_y
                        # ring: starts while the last projection windows
                        # still hold ps_proj
                        for tk in range(2):
                            k0 = 128 * tk
                            for r in range(2):
                                kh = ka[64 * r:64 * (r + 1), :]
                                qh = qa[64 * r:64 * (r + 1), :]
                                ps = ps_y.tile([128, 512], F32,
                                               name="ps_qk0", tag="ps_y")
                                nc.tensor.matmul(
                                    ps[:, k0:512], kh[:, k0:k0 + 128],
                                    qh[:, k0:512], start=True, stop=True,
                                )
                                nc.vector.scalar_tensor_tensor(
                                    att[tk][:, r, k0:512], ps[:, k0:512],
                                    0.0, masks_sb[:, 0, :512 - k0],
                                    AluOpType.max, AluOpType.mult,
                                )
                                ps = ps_y.tile([128, 512], F32,
                                               name="ps_qk0", tag="ps_y")
                                nc.tensor.matmul(
                                    ps[:], kh[:, k0:k0 + 128], qh[:, 512:T],
                                    start=True, stop=True,
                                )
                                nc.scalar.activation(
                                    att[tk][:, r, 512:T], ps[:], AF.Relu)
                    # diag-window pieces tk4-7 first (bank-high AV deps),
                    # then the [512:T) halves of tk0-3, then the diagonal
                    # halves of tk0-3 (bank-low AV deps, resolved last)
                    qk_piece(att, qa, ka, 4, 512, T, "mix")
                    qk_piece(att, qa, ka, 5, 640, T, "dve")
                    qk_piece(att, qa, ka, 6, 768, T, "dve")
                    qk_piece(att, qa, ka, 7, 896, T, "dve")
                    lo_tks = range(2, 4) if a == 0 else range(4)
                    for tk in lo_tks:
                        qk_piece(att, qa, ka, tk, 512, T, "act")
                    for tk in lo_tks:
                        qk_piece(att, qa, ka, tk, 128 * tk, 512, "dve")

                    # ---- AV: y[q, d] per q-tile, att.T block stationary,
                    # v columns moving; both heads pack one [128, 512]
                    # bank: cols = 128*(t%4) + 64*r + d. Bank-high first
                    # (deps ready mid-pair), bank-low last. ----
                    for bk in (1, 0):
                        ps2 = ps_y.tile([128, 512], F32, name="ps_av",
                                        tag="ps_y")
                        y_sb = yop.tile([128, 512], F16, name="y_sb",
                                        tag="y_sb")
                        for t in range(4 * bk, 4 * bk + 4):
                            for r in range(2):
                                h = 2 * a + r
                                col = 128 * (t % 4) + 64 * r
                                for k in range(t + 1):
                                    nc.tensor.matmul(
                                        ps2[:, col:col + 64],
                                        att[k][:, r, 128 * t:128 * (t + 1)],
                                        v_sb[k][:, 64 * h:64 * (h + 1)],
                                        start=(k == 0), stop=(k == t),
                                    )
                        if bk == 1:
                            nc.scalar.copy(y_sb[:], ps2[:])
                            nc.sync.dma_start(
                                out=y_d[4:8, :, 128 * a:128 * (a + 1)]
                                    .transpose([1, 0, 2]),
                                in_=y_sb[:])
                        else:
                            # split eviction so the closing chain after the
                            # last matmul is one [128, 256] copy + small DMA
                            nc.vector.tensor_scalar(
                                y_sb[:, 0:256], ps2[:, 0:256], 0.0, None,
                                AluOpType.add)
                            nc.sync.dma_start(
                                out=y_d[0:2, :, 128 * a:128 * (a + 1)]
                                    .transpose([1, 0, 2]),
                                in_=y_sb[:, 0:256])
                            nc.scalar.copy(y_sb[:, 256:512], ps2[:, 256:512])
                            nc.sync.dma_start(
                                out=y_d[2:4, :, 128 * a:128 * (a + 1)]
                                    .transpose([1, 0, 2]),
                                in_=y_sb[:, 256:512])

    nc.compile()
    return nc


def _prep_host(x, W_attn, b_attn):
    s = 1.0 / np.sqrt(np.float32(HD))
    W = np.asarray(W_attn, dtype=np.float32).copy()
    b = np.asarray(b_attn, dtype=np.float32).copy()
    W[:C] *= s
    b[:C] *= s
    # interleave q/k head pairs: [q-pair0, k-pair0, q-pair1, k-pair1, ...], v natural
    rows = []
    for a in range(NPAIR):
        rows.extend(range(128 * a, 128 * (a + 1)))          # q heads 2a, 2a+1
        rows.extend(range(C + 128 * a, C + 128 * (a + 1)))  # k heads 2a, 2a+1
    rows.extend(range(2 * C, 3 * C))                        # v natural
    W_perm = W[rows]
    b_perm = b[rows]

    e4 = ml_dtypes.float8_e4m3

    def pack(mat):
        # (C, N) -> partition-major (128, KT, N): each partition's six
        # contraction k-tiles contiguous, k-pair-major
        Cr, N = mat.shape
        return np.ascontiguousarray(
            mat.reshape(KT, 128, N).transpose(1, 0, 2))

    def split8(mat):
        hi = mat.astype(e4)
        lo = (mat - hi.astype(np.float32)).astype(e4)
        return hi, lo

    wT = np.ascontiguousarray(W_perm.T) * np.float32(SW)     # (C, 3C)
    wqh, wql = split8(pack(wT[:, :2 * C]))
    wvh, wvl = split8(pack(wT[:, 2 * C:]))
    bqk = np.ascontiguousarray(b_perm[:2 * C].reshape(2 * NPAIR, 128).T)  # (128, 12)
    bvb = np.ascontiguousarray(
        np.broadcast_to(b_perm[2 * C:], (128, C))).astype(np.float16)
    tri = (np.arange(128)[None, :] >= np.arange(128)[:, None]).astype(np.float32)
    m1 = np.ones((128, T), dtype=np.float32)
    m1[:, 0:128] = tri             # kept windows always start at the diagonal
    masks = np.ascontiguousarray(
        np.broadcast_to(m1[:, None, :], (128, 2, T)))
    xT = np.asarray(x, dtype=np.float32).transpose(0, 2, 1) * np.float32(SX)  # (B, C, T)
    xhv = np.stack([pack(xT[c]) for c in range(B)])
    xhv, xlv = split8(xhv)
    return xhv, xlv, wqh, wql, wvh, wvl, bqk, bvb, masks


def kernel(x, W_attn, b_attn):
    if "nc" not in _CACHE:
        _CACHE["nc"] = _build()
    nc = _CACHE["nc"]

    (xhv, xlv, wqh, wql, wvh, wvl, bqk, bvb, masks) = _prep_host(x, W_attn, b_attn)
    in_maps = [
        {"xh": xhv[c], "xl": xlv[c], "wqh": wqh, "wql": wql, "wvh": wvh,
         "wvl": wvl, "bqk": bqk, "bvb": bvb, "masks": masks}
        for c in range(B)
    ]
    res = run_bass_kernel_spmd(nc, in_maps, list(range(B)))
    y = np.empty((B, T, C), dtype=np.float32)
    for c in range(B):
        y[c] = res.results[c]["y"].reshape(T, C).astype(np.float32)
    return y


# revision 5
# speedup vs baseline: 1.0432x; 1.0432x over previous
"""Trainium2 Bass kernel for causal masked-ReLU attention (no softmax).

Reference computation (B=8, T=1024, C=768, n_head=12, hd=64):
    qkv = x @ W_attn.T + b_attn
    q, k, v = split(qkv); per-head: att = relu(mask_causal(q k^T / sqrt(hd)))
    y = att @ v, heads re-merged -> (B, T, C)

Sharding: one batch element per NeuronCore (8 cores). Each core computes the
QKV projection and all 12 heads' attention for its batch element.

Layout strategy (per core):
  - Host passes x[b].T (C, T) and W.T (C, 3C) so the contraction dim C lands
    on SBUF partitions with unit-stride DMA (no on-chip transposes).
  - W rows are pre-permuted on host into [q-pair0, k-pair0, q-pair1, ...] so
    q.T / k.T of head h live at the same partition offset (h%2)*64 of their
    M-tiles; q weights/bias pre-scaled by 1/sqrt(hd).
  - QKV projection runs in fp8 (e4m3) DoubleRow perf mode: 256-deep
    contraction per pass at 0.5 cycles/row. Operands split into hi+lo fp8
    digits and three digit products xh*wh + xl*wh + xh*wl accumulate in one
    fp32 PSUM group (dropped xl*wl ~1e-4 relative); the 2^13 operand scale
    comes out at eviction with the bias. PE cost 0.75x of fp16.
  - att is computed transposed (att.T = k @ q.T, layout [T_k, T_q]), fp16
    everywhere, BOTH heads of a pair sharing one [128, 2, 512] double-bank
    PSUM tile and one [128, 2, T] SBUF tile per k-tile: the per-instruction
    PSUM-access penalty on DVE/ACT (~125/185ns) dominates phase 2, and
    pair-merging halves the eviction op count.
  - att tiles are double-buffered ACROSS pairs (sets alternate a%2): pair
    a+1's QK pieces stream through DVE/ACT while pair a's AV still reads
    the other set, so the mask-bound DVE runs and relu-bound ACT runs
    interleave instead of convoying (Pool cannot read PSUM, so evictions
    have only these two engines).
  - Per-pair piece order tk4..tk7, then [512:T) halves of tk0-3, then
    diagonal halves of tk0-3: AV bank-high's dependencies resolve
    mid-pair and bank-low's at the end, so the closing chain after the
    final AV matmul is one [128, 256] copy + one small DMA.
  - The AV product exploits weight-stationary asymmetry: per (q-tile,
    k-tile, head) matmul the STATIONARY operand is the [128, 128] att.T
    block and the MOVING operand is the head's 64 v columns (Ldweights is
    free; cost follows moving columns). Both heads accumulate into one
    [128, 512] PSUM bank (cols = 128*(t%4) + 64*head_parity + d); y
    DMAs out with a 3-d strided AP into natural (T, C) layout.
  - Causal masking at eviction via a [tri(128) | ones] relu-mask; DVE owns
    mask-needing pieces (only PSUM-capable engine with
    scalar_tensor_tensor), ACT the pure-relu halves; tile 4 splits
    [masked 2x128 | relu 2x384] to converge both at ~6.0us/pair, just
    above PE's 5.8us/pair.
  - The input front is split fine (x column halves) and spread across the
    SP HWDGE queue and the Pool SWDGE queue (which bypasses the shared
    HWDGE's ~625ns/DMA): the first window's eviction gates on all three
    contraction pairs, and a single queue starves the PE ~3us.
  - Warmup matmuls on a zeroed scratch tile ramp the PE p-state during the
    initial DMA wait, with more sprinkled into the first windows' stall
    points (idle gaps reset the ~3us ramp).
  - Projection windows: 2 groups per double-bank tile, 3 tiles rotating;
    mid-stream windows borrow the two spare ps_y banks for 8 groups in
    flight. One pool scope spans both phases (pool close = all-engine
    barrier); pair 0's first two k-tiles run unmerged through ps_y so
    phase 2 overlaps the last projection windows.
  - Output is written as y (T, C) in fp16; host upcasts.
"""

import numpy as np

import sys
for _p in ("/opt/trn_rl_repo", "/root/.axon_site", "/root/.axon_site/_ro/trn_rl_repo",
           "/root/.axon_site/_ro/pypackages"):
    if _p not in sys.path:
        sys.path.append(_p)

import ml_dtypes

import concourse.bacc as bacc
import concourse.mybir as mybir
from concourse.alu_op_type import AluOpType
from concourse.tile import TileContext
from concourse.bass_utils import run_bass_kernel_spmd

B, T, C = 8, 1024, 768
NH, HD = 12, 64
C3 = 3 * C            # 2304
KT = C // 128         # 6  contraction tiles of the projection
NP = KT // 2          # 3  contraction pairs (DoubleRow)
TT = T // 128         # 8  tiles of the sequence dim
NPAIR = NH // 2       # 6  head pairs
F32 = mybir.dt.float32
F16 = mybir.dt.float16
F8 = mybir.dt.float8e4
AF = mybir.ActivationFunctionType
DR = mybir.MatmulPerfMode.DoubleRow

SX = 16.0             # x pre-scale (keeps x-lo digits in e4m3 normal range)
SW = 512.0            # W pre-scale
DESCALE = 1.0 / (SX * SW)

# warmup matmul moving-widths (fp16): ramp the PE p-state during the
# initial input-DMA wait so real matmuls start at full clock
WARM = [512] * 4 + [256]
NSPRINKLE = 4         # 256-wide ramp-guard warmups per early stall point

_CACHE = {}


def _build():
    nc = bacc.Bacc("TRN2", target_bir_lowering=False, debug=False, num_devices=8)

    xh = nc.dram_tensor("xh", [128, KT, T], F8, kind="ExternalInput").ap()
    xl = nc.dram_tensor("xl", [128, KT, T], F8, kind="ExternalInput").ap()
    wvh = nc.dram_tensor("wvh", [128, KT, C], F8, kind="ExternalInput").ap()
    wvl = nc.dram_tensor("wvl", [128, KT, C], F8, kind="ExternalInput").ap()
    wqh = nc.dram_tensor("wqh", [128, KT, 2 * C], F8, kind="ExternalInput").ap()
    wql = nc.dram_tensor("wql", [128, KT, 2 * C], F8, kind="ExternalInput").ap()
    bqk = nc.dram_tensor("bqk", [128, 2 * NPAIR], F32, kind="ExternalInput").ap()
    bvb = nc.dram_tensor("bvb", [128, C], F16, kind="ExternalInput").ap()
    # masks = [tri(128) | ones(896)] duplicated along dim1 so pair-merged
    # [128, 2, W] evictions read the same relu-mask for both heads
    masks = nc.dram_tensor("masks", [128, 2, T], F32, kind="ExternalInput").ap()
    # y in natural (T, C) layout, tiled (TT, 128, C) for the 3-d AV DMAs
    y_d = nc.dram_tensor("y", [TT, 128, C], F16, kind="ExternalOutput").ap()

    with TileContext(nc) as tc:
        with (
            tc.tile_pool(name="persist", bufs=1) as pp,
        ):
            masks_sb = pp.tile([128, 2, T], F32, name="masks_sb")
            bqk_sb = pp.tile([128, 2 * NPAIR], F32, name="bqk_sb")
            bvb_sb = pp.tile([128, C], F16, name="bvb_sb")
            qkT = [pp.tile([128, T], F16, name=f"qkT{m}") for m in range(2 * NPAIR)]
            v_sb = [pp.tile([128, C], F16, name=f"v{t}") for t in range(TT)]
            # att.T tiles, fp16, dim1 = head parity; two sets alternating
            # per pair so pair a+1's QK overlaps pair a's AV
            att2 = [[pp.tile([128, 2, T], F16, name=f"att{s}_{t}")
                     for t in range(TT)] for s in range(2)]

            # ---------- Phase 1: QKV projection (fp8 DoubleRow, 3 digit
            # products into one PSUM group) ----------
            from contextlib import ExitStack
            with ExitStack() as stack:
                iop = stack.enter_context(tc.tile_pool(name="io", bufs=1))
                # 3 double-bank tiles (12KB/partition) shared by projection
                # windows and phase-2 merged QK pieces; + 2 single banks for
                # warmup / window-borrow / early-QK / AV
                ps_proj = stack.enter_context(
                    tc.tile_pool(name="psum_proj", bufs=3, space="PSUM"))
                ps_y = stack.enter_context(
                    tc.tile_pool(name="psum_y", bufs=2, space="PSUM"))
                yop = stack.enter_context(tc.tile_pool(name="yout", bufs=2))
                xh_sb = iop.tile([128, KT, T], F8, name="xh_sb")
                xl_sb = iop.tile([128, KT, T], F8, name="xl_sb")
                wv_h = iop.tile([128, KT, C], F8, name="wv_h")
                wv_l = iop.tile([128, KT, C], F8, name="wv_l")
                wq_h = iop.tile([128, KT, 2 * C], F8, name="wq_h")
                wq_l = iop.tile([128, KT, 2 * C], F8, name="wq_l")

                # PE p-state warmup on a zeroed scratch tile; results are
                # never read
                scratch = iop.tile([128, 512], F16, name="warm_src")
                nc.vector.memset(scratch[:], 0.0)
                warm = ps_y.tile([128, 512], F32, name="ps_warm", tag="ps_y")
                for w in WARM:
                    nc.tensor.matmul(warm[:, :w], scratch[:, :128],
                                     scratch[:, :w], start=True, stop=True)

                # input DMAs. The first windows' evictions gate on ALL three
                # contraction pairs, so the front ships x in column halves
                # and splits across the SP HWDGE queue and the Pool SWDGE
                # queue (parallel descriptor generators).
                sp, pool = nc.sync, nc.gpsimd
                sp.dma_start(out=wv_h[:, 0:2, :], in_=wvh[:, 0:2, :])
                sp.dma_start(out=xh_sb[:, 0:2, 0:256], in_=xh[:, 0:2, 0:256])
                pool.dma_start(out=xl_sb[:, 0:2, 0:512], in_=xl[:, 0:2, 0:512])
                sp.dma_start(out=wv_l[:, 0:2, :], in_=wvl[:, 0:2, :])
                sp.dma_start(out=xh_sb[:, 0:2, 256:512], in_=xh[:, 0:2, 256:512])
                pool.dma_start(out=xh_sb[:, 2:4, 0:512], in_=xh[:, 2:4, 0:512])
                sp.dma_start(out=wv_h[:, 2:4, :], in_=wvh[:, 2:4, :])
                pool.dma_start(out=xl_sb[:, 2:4, 0:512], in_=xl[:, 2:4, 0:512])
                sp.dma_start(out=wv_l[:, 2:4, :], in_=wvl[:, 2:4, :])
                pool.dma_start(out=xh_sb[:, 4:6, 0:512], in_=xh[:, 4:6, 0:512])
                sp.dma_start(out=wv_h[:, 4:6, :], in_=wvh[:, 4:6, :])
                pool.dma_start(out=xl_sb[:, 4:6, 0:512], in_=xl[:, 4:6, 0:512])
                sp.dma_start(out=wv_l[:, 4:6, :], in_=wvl[:, 4:6, :])
                sp.dma_start(out=bvb_sb[:], in_=bvb[:])
                # x column rests (v tiles 4-7 of the seq dim; windows 2-3)
                for p in range(NP):
                    pr = slice(2 * p, 2 * p + 2)
                    sp.dma_start(out=xh_sb[:, pr, 512:T], in_=xh[:, pr, 512:T])
                    pool.dma_start(out=xl_sb[:, pr, 512:T], in_=xl[:, pr, 512:T])
                # q/k weights; pair 0 in m0-m3 / m4-m11 halves
                pr0 = slice(0, 2)
                sp.dma_start(out=wq_h[:, pr0, :512], in_=wqh[:, pr0, :512])
                pool.dma_start(out=wq_l[:, pr0, :512], in_=wql[:, pr0, :512])
                sp.dma_start(out=wq_h[:, pr0, 512:], in_=wqh[:, pr0, 512:])
                pool.dma_start(out=wq_l[:, pr0, 512:], in_=wql[:, pr0, 512:])
                sp.dma_start(out=bqk_sb[:], in_=bqk[:])
                for p in range(1, NP):
                    prp = slice(2 * p, 2 * p + 2)
                    sp.dma_start(out=wq_h[:, prp, :], in_=wqh[:, prp, :])
                    pool.dma_start(out=wq_l[:, prp, :], in_=wql[:, prp, :])
                sp.dma_start(out=masks_sb[:], in_=masks[:])

                # each group = one [128, 512] PSUM bank lane.
                # ("v", t, n0, width) / ("qk", m, q0, width)
                groups = []
                for t in range(TT):
                    groups.append(("v", t, 0, 512))
                    groups.append(("v", t, 512, 256))
                for m in range(2 * NPAIR):
                    for q0 in (0, 512):
                        groups.append(("qk", m, q0, 512))

                # windows of 4 groups; k-pair-major, digit-product-minor so
                # PE consumption order matches DMA arrival order. Mid-stream
                # windows borrow the two ps_y banks for deeper pipelining.
                nwin = (len(groups) + 3) // 4
                for wi, w0 in enumerate(range(0, len(groups), 4)):
                    window = groups[w0:w0 + 4]
                    borrow = 3 <= wi < nwin - 2
                    if borrow:
                        dbl = ps_proj.tile([128, 2, 512], F32,
                                           name="ps_proj", tag="ps_proj")
                        tiles = [dbl[:, 0, :], dbl[:, 1, :],
                                 ps_y.tile([128, 512], F32, name="ps_b",
                                           tag="ps_y"),
                                 ps_y.tile([128, 512], F32, name="ps_b",
                                           tag="ps_y")][:len(window)]
                    else:
                        dbl = [ps_proj.tile([128, 2, 512], F32,
                                            name="ps_proj", tag="ps_proj")
                               for _ in range((len(window) + 1) // 2)]
                        tiles = [dbl[gi // 2][:, gi % 2, :]
                                 for gi in range(len(window))]
                    nmm = [0] * len(window)
                    total = [9 * (g[3] // 256) for g in window]
                    for p in range(NP):
                        pr = slice(2 * p, 2 * p + 2)
                        for term in range(3):
                            xa = (xh_sb, xl_sb, xh_sb)[term]
                            wva = (wv_h, wv_h, wv_l)[term]
                            wqa = (wq_h, wq_h, wq_l)[term]
                            for gi, (g, ps) in enumerate(zip(window, tiles)):
                                kind, i, o0, wd = g
                                for c0 in range(0, wd, 256):
                                    n = nmm[gi]
                                    nmm[gi] = n + 1
                                    st = n == 0
                                    sp_ = n == total[gi] - 1
                                    if kind == "v":
                                        nc.tensor.matmul(
                                            ps[:, c0:c0 + 256],
                                            xa[:, pr, 128 * i:128 * (i + 1)],
                                            wva[:, pr, o0 + c0:o0 + c0 + 256],
                                            start=st, stop=sp_, perf_mode=DR,
                                        )
                                    else:
                                        nc.tensor.matmul(
                                            ps[:, c0:c0 + 256],
                                            wqa[:, pr, 128 * i:128 * (i + 1)],
                                            xa[:, pr, o0 + c0:o0 + c0 + 256],
                                            start=st, stop=sp_, perf_mode=DR,
                                        )
                            if wi < 2 and term == 2:
                                # ramp-guard warmups at the early stall
                                # points (pair p+1 still in flight)
                                for _ in range(NSPRINKLE):
                                    nc.tensor.matmul(
                                        warm[:, :256], scratch[:, :128],
                                        scratch[:, :256], start=True,
                                        stop=True)
                    for g, ps in zip(window, tiles):
                        kind, i, o0, wd = g
                        if kind == "v":
                            nc.vector.scalar_tensor_tensor(
                                v_sb[i][:, o0:o0 + wd], ps[:, :wd], DESCALE,
                                bvb_sb[:, o0:o0 + wd],
                                AluOpType.mult, AluOpType.add,
                            )
                        elif i % 2 == 0:
                            nc.scalar.activation(
                                qkT[i][:, o0:o0 + wd], ps[:, :wd],
                                AF.Identity, bias=bqk_sb[:, i:i + 1],
                                scale=DESCALE,
                            )
                        else:
                            nc.vector.tensor_scalar(
                                qkT[i][:, o0:o0 + wd], ps[:, :wd],
                                DESCALE, bqk_sb[:, i:i + 1],
                                AluOpType.mult, AluOpType.add,
                            )

            # ---------- Phase 2: attention, pair by pair (same pool
            # scope: no phase barrier) ----------
                def qk_piece(att, qa, ka, tk, lo, hi, engine):
                    """One pair-merged QK piece covering q cols [lo, hi);
                    engine: 'dve' masked relu / 'act' pure relu / 'mix'
                    masked first 128 cols on DVE + relu rest on ACT."""
                    k0 = 128 * tk
                    ps = ps_proj.tile([128, 2, 512], F32, name="ps_qk",
                                      tag="ps_proj")
                    for r in range(2):
                        nc.tensor.matmul(
                            ps[:, r, :hi - lo],
                            ka[64 * r:64 * (r + 1), k0:k0 + 128],
                            qa[64 * r:64 * (r + 1), lo:hi],
                            start=True, stop=True,
                        )
                    if engine == "act":
                        nc.scalar.activation(att[tk][:, :, lo:hi],
                                             ps[:, :, :hi - lo], AF.Relu)
                    elif engine == "dve":
                        nc.vector.scalar_tensor_tensor(
                            att[tk][:, :, lo:hi], ps[:, :, :hi - lo],
                            0.0, masks_sb[:, :, :hi - lo],
                            AluOpType.max, AluOpType.mult,
                        )
                    else:  # mix: diag block on DVE, remainder on ACT
                        nc.vector.scalar_tensor_tensor(
                            att[tk][:, :, lo:lo + 128], ps[:, :, 0:128],
                            0.0, masks_sb[:, :, :128],
                            AluOpType.max, AluOpType.mult,
                        )
                        nc.scalar.activation(att[tk][:, :, lo + 128:hi],
                                             ps[:, :, 128:hi - lo], AF.Relu)

                for a in range(NPAIR):
                    att = att2[a % 2]
                    qa, ka = qkT[2 * a], qkT[2 * a + 1]
                    if a == 0:
                        # tk0/tk1 as unmerged singles through the spare ps_y
                        # ring: starts while the last projection windows
                        # still hold ps_proj
                        for tk in range(2):
                            k0 = 128 * tk
                            for r in range(2):
                                kh = ka[64 * r:64 * (r + 1), :]
                                qh = qa[64 * r:64 * (r + 1), :]
                                ps = ps_y.tile([128, 512], F32,
                                               name="ps_qk0", tag="ps_y")
                                nc.tensor.matmul(
                                    ps[:, k0:512], kh[:, k0:k0 + 128],
                                    qh[:, k0:512], start=True, stop=True,
                                )
                                nc.vector.scalar_tensor_tensor(
                                    att[tk][:, r, k0:512], ps[:, k0:512],
                                    0.0, masks_sb[:, 0, :512 - k0],
                                    AluOpType.max, AluOpType.mult,
                                )
                                ps = ps_y.tile([128, 512], F32,
                                               name="ps_qk0", tag="ps_y")
                                nc.tensor.matmul(
                                    ps[:], kh[:, k0:k0 + 128], qh[:, 512:T],
                                    start=True, stop=True,
                                )
                                nc.scalar.activation(
                                    att[tk][:, r, 512:T], ps[:], AF.Relu)
                    # piece order alternates DVE-evicted (masked) and
                    # ACT-evicted (pure-relu) pieces: the 3-slot psum ring
                    # recycles FIFO, so same-engine runs convoy both engines.
                    # tk5-7 early (bank-high deps), p1 halves last (bank-low
                    # deps resolve at pair end, keeping the tail chain short)
                    if a == 0:
                        order = [(5, 640, T, "dve"), (2, 512, T, "act"),
                                 (6, 768, T, "dve"), (3, 512, T, "act"),
                                 (7, 896, T, "dve"), (2, 256, 512, "dve"),
                                 (4, 512, T, "mix"), (3, 384, 512, "dve")]
                    else:
                        order = [(5, 640, T, "dve"), (0, 512, T, "act"),
                                 (6, 768, T, "dve"), (1, 512, T, "act"),
                                 (7, 896, T, "dve"), (2, 512, T, "act"),
                                 (0, 0, 512, "dve"), (3, 512, T, "act"),
                                 (1, 128, 512, "dve"), (4, 512, T, "mix"),
                                 (2, 256, 512, "dve"), (3, 384, 512, "dve")]
                    for tk, lo, hi, eng in order:
                        qk_piece(att, qa, ka, tk, lo, hi, eng)

                    # ---- AV: y[q, d] per q-tile, att.T block stationary,
                    # v columns moving; both heads pack one [128, 512]
                    # bank: cols = 128*(t%4) + 64*r + d. Bank-high first
                    # (deps ready mid-pair), bank-low last. ----
                    for bk in (1, 0):
                        ps2 = ps_y.tile([128, 512], F32, name="ps_av",
                                        tag="ps_y")
                        y_sb = yop.tile([128, 512], F16, name="y_sb",
                                        tag="y_sb")
                        for t in range(4 * bk, 4 * bk + 4):
                            for r in range(2):
                                h = 2 * a + r
                                col = 128 * (t % 4) + 64 * r
                                for k in range(t + 1):
                                    nc.tensor.matmul(
                                        ps2[:, col:col + 64],
                                        att[k][:, r, 128 * t:128 * (t + 1)],
                                        v_sb[k][:, 64 * h:64 * (h + 1)],
                                        start=(k == 0), stop=(k == t),
                                    )
                        if bk == 1:
                            nc.scalar.copy(y_sb[:], ps2[:])
                            nc.sync.dma_start(
                                out=y_d[4:8, :, 128 * a:128 * (a + 1)]
                                    .transpose([1, 0, 2]),
                                in_=y_sb[:])
                        else:
                            # split eviction so the closing chain after the
                            # last matmul is one [128, 256] copy + small DMA
                            nc.vector.tensor_scalar(
                                y_sb[:, 0:256], ps2[:, 0:256], 0.0, None,
                                AluOpType.add)
                            nc.sync.dma_start(
                                out=y_d[0:2, :, 128 * a:128 * (a + 1)]
                                    .transpose([1, 0, 2]),
                                in_=y_sb[:, 0:256])
                            nc.scalar.copy(y_sb[:, 256:512], ps2[:, 256:512])
                            nc.sync.dma_start(
                                out=y_d[2:4, :, 128 * a:128 * (a + 1)]
                                    .transpose([1, 0, 2]),
                                in_=y_sb[:, 256:512])

    nc.compile()
    return nc


def _prep_host(x, W_attn, b_attn):
    s = 1.0 / np.sqrt(np.float32(HD))
    W = np.asarray(W_attn, dtype=np.float32).copy()
    b = np.asarray(b_attn, dtype=np.float32).copy()
    W[:C] *= s
    b[:C] *= s
    # interleave q/k head pairs: [q-pair0, k-pair0, q-pair1, k-pair1, ...], v natural
    rows = []
    for a in range(NPAIR):
        rows.extend(range(128 * a, 128 * (a + 1)))          # q heads 2a, 2a+1
        rows.extend(range(C + 128 * a, C + 128 * (a + 1)))  # k heads 2a, 2a+1
    rows.extend(range(2 * C, 3 * C))                        # v natural
    W_perm = W[rows]
    b_perm = b[rows]

    e4 = ml_dtypes.float8_e4m3

    def pack(mat):
        # (C, N) -> partition-major (128, KT, N): each partition's six
        # contraction k-tiles contiguous, k-pair-major
        Cr, N = mat.shape
        return np.ascontiguousarray(
            mat.reshape(KT, 128, N).transpose(1, 0, 2))

    def split8(mat):
        hi = mat.astype(e4)
        lo = (mat - hi.astype(np.float32)).astype(e4)
        return hi, lo

    wT = np.ascontiguousarray(W_perm.T) * np.float32(SW)     # (C, 3C)
    wqh, wql = split8(pack(wT[:, :2 * C]))
    wvh, wvl = split8(pack(wT[:, 2 * C:]))
    bqk = np.ascontiguousarray(b_perm[:2 * C].reshape(2 * NPAIR, 128).T)  # (128, 12)
    bvb = np.ascontiguousarray(
        np.broadcast_to(b_perm[2 * C:], (128, C))).astype(np.float16)
    tri = (np.arange(128)[None, :] >= np.arange(128)[:, None]).astype(np.float32)
    m1 = np.ones((128, T), dtype=np.float32)
    m1[:, 0:128] = tri             # kept windows always start at the diagonal
    masks = np.ascontiguousarray(
        np.broadcast_to(m1[:, None, :], (128, 2, T)))
    xT = np.asarray(x, dtype=np.float32).transpose(0, 2, 1) * np.float32(SX)  # (B, C, T)
    xhv = np.stack([pack(xT[c]) for c in range(B)])
    xhv, xlv = split8(xhv)
    return xhv, xlv, wqh, wql, wvh, wvl, bqk, bvb, masks


def kernel(x, W_attn, b_attn):
    if "nc" not in _CACHE:
        _CACHE["nc"] = _build()
    nc = _CACHE["nc"]

    (xhv, xlv, wqh, wql, wvh, wvl, bqk, bvb, masks) = _prep_host(x, W_attn, b_attn)
    in_maps = [
        {"xh": xhv[c], "xl": xlv[c], "wqh": wqh, "wql": wql, "wvh": wvh,
         "wvl": wvl, "bqk": bqk, "bvb": bvb, "masks": masks}
        for c in range(B)
    ]
    res = run_bass_kernel_spmd(nc, in_maps, list(range(B)))
    y = np.empty((B, T, C), dtype=np.float32)
    for c in range(B):
        y[c] = res.results[c]["y"].reshape(T, C).astype(np.float32)
    return y


# revision 6
# speedup vs baseline: 1.0999x; 1.0543x over previous
"""Trainium2 Bass kernel for causal masked-ReLU attention (no softmax).

Reference computation (B=8, T=1024, C=768, n_head=12, hd=64):
    qkv = x @ W_attn.T + b_attn
    q, k, v = split(qkv); per-head: att = relu(mask_causal(q k^T / sqrt(hd)))
    y = att @ v, heads re-merged -> (B, T, C)

Sharding: one batch element per NeuronCore (8 cores). Each core computes the
QKV projection and all 12 heads' attention for its batch element.

Layout strategy (per core):
  - Host passes x[b].T (C, T) and W.T (C, 3C) so the contraction dim C lands
    on SBUF partitions with unit-stride DMA (no on-chip transposes).
  - W rows are pre-permuted on host into [q-pair0, k-pair0, q-pair1, ...] so
    q.T / k.T of head h live at the same partition offset (h%2)*64 of their
    M-tiles; q weights/bias pre-scaled by 1/sqrt(hd).
  - QKV projection runs in fp8 (e4m3) DoubleRow perf mode: 256-deep
    contraction per pass at 0.5 cycles/row. Operands split into hi+lo fp8
    digits and three digit products xh*wh + xl*wh + xh*wl accumulate in one
    fp32 PSUM group (dropped xl*wl ~1e-4 relative); the 2^13 operand scale
    comes out at eviction with the bias. PE cost 0.75x of fp16.
  - att is computed transposed (att.T = k @ q.T, layout [T_k, T_q]), fp16
    everywhere, BOTH heads of a pair sharing one [128, 2, 512] double-bank
    PSUM tile and one [128, 2, T] SBUF tile per k-tile: the per-instruction
    PSUM-access penalty on DVE/ACT (~125/185ns) dominates phase 2, and
    pair-merging halves the eviction op count.
  - att tiles are double-buffered ACROSS pairs (sets alternate a%2): pair
    a+1's QK pieces stream through DVE/ACT while pair a's AV still reads
    the other set, so the mask-bound DVE runs and relu-bound ACT runs
    interleave instead of convoying (Pool cannot read PSUM, so evictions
    have only these two engines).
  - Per-pair piece order tk4..tk7, then [512:T) halves of tk0-3, then
    diagonal halves of tk0-3: AV bank-high's dependencies resolve
    mid-pair and bank-low's at the end, so the closing chain after the
    final AV matmul is one [128, 256] copy + one small DMA.
  - The AV product exploits weight-stationary asymmetry: per (q-tile,
    k-tile, head) matmul the STATIONARY operand is the [128, 128] att.T
    block and the MOVING operand is the head's 64 v columns (Ldweights is
    free; cost follows moving columns). Both heads accumulate into one
    [128, 512] PSUM bank (cols = 128*(t%4) + 64*head_parity + d); y
    DMAs out with a 3-d strided AP into natural (T, C) layout.
  - Causal masking at eviction via a [tri(128) | ones] relu-mask; DVE owns
    mask-needing pieces (only PSUM-capable engine with
    scalar_tensor_tensor), ACT the pure-relu halves; tile 4 splits
    [masked 2x128 | relu 2x384] to converge both at ~6.0us/pair, just
    above PE's 5.8us/pair.
  - The input front is split fine (x column halves) and spread across the
    SP HWDGE queue and the Pool SWDGE queue (which bypasses the shared
    HWDGE's ~625ns/DMA): the first window's eviction gates on all three
    contraction pairs, and a single queue starves the PE ~3us.
  - Warmup matmuls on a zeroed scratch tile ramp the PE p-state during the
    initial DMA wait, with more sprinkled into the first windows' stall
    points (idle gaps reset the ~3us ramp).
  - Projection windows: 2 groups per double-bank tile, 3 tiles rotating;
    mid-stream windows borrow the two spare ps_y banks for 8 groups in
    flight. One pool scope spans both phases (pool close = all-engine
    barrier); pair 0's first two k-tiles run unmerged through ps_y so
    phase 2 overlaps the last projection windows.
  - Output is written as y (T, C) in fp16; host upcasts.
"""

import numpy as np

import sys
for _p in ("/opt/trn_rl_repo", "/root/.axon_site", "/root/.axon_site/_ro/trn_rl_repo",
           "/root/.axon_site/_ro/pypackages"):
    if _p not in sys.path:
        sys.path.append(_p)

import ml_dtypes

import concourse.bacc as bacc
import concourse.mybir as mybir
from concourse.alu_op_type import AluOpType
from concourse.tile import TileContext
from concourse.bass_utils import run_bass_kernel_spmd

B, T, C = 8, 1024, 768
NH, HD = 12, 64
C3 = 3 * C            # 2304
KT = C // 128         # 6  contraction tiles of the projection
NP = KT // 2          # 3  contraction pairs (DoubleRow)
TT = T // 128         # 8  tiles of the sequence dim
NPAIR = NH // 2       # 6  head pairs
F32 = mybir.dt.float32
F16 = mybir.dt.float16
F8 = mybir.dt.float8e4
AF = mybir.ActivationFunctionType
DR = mybir.MatmulPerfMode.DoubleRow

SX = 16.0             # x pre-scale (keeps x-lo digits in e4m3 normal range)
SW = 512.0            # W pre-scale
DESCALE = 1.0 / (SX * SW)

# warmup matmul moving-widths (fp16): ramp the PE p-state during the
# initial input-DMA wait so real matmuls start at full clock
WARM = [512] * 4 + [256]
NSPRINKLE = 4         # 256-wide ramp-guard warmups per early stall point

_CACHE = {}


def _build():
    nc = bacc.Bacc("TRN2", target_bir_lowering=False, debug=False, num_devices=8)

    xh = nc.dram_tensor("xh", [128, KT, T], F8, kind="ExternalInput").ap()
    xl = nc.dram_tensor("xl", [128, KT, T], F8, kind="ExternalInput").ap()
    wvh = nc.dram_tensor("wvh", [128, KT, C], F8, kind="ExternalInput").ap()
    wvl = nc.dram_tensor("wvl", [128, KT, C], F8, kind="ExternalInput").ap()
    wqh = nc.dram_tensor("wqh", [128, KT, 2 * C], F8, kind="ExternalInput").ap()
    wql = nc.dram_tensor("wql", [128, KT, 2 * C], F8, kind="ExternalInput").ap()
    bqk = nc.dram_tensor("bqk", [128, 2 * NPAIR], F32, kind="ExternalInput").ap()
    bvb = nc.dram_tensor("bvb", [128, C], F16, kind="ExternalInput").ap()
    # masks = [tri(128) | ones(896)] duplicated along dim1 so pair-merged
    # [128, 2, W] evictions read the same relu-mask for both heads
    masks = nc.dram_tensor("masks", [128, 2, T], F32, kind="ExternalInput").ap()
    # y in natural (T, C) layout, tiled (TT, 128, C) for the 3-d AV DMAs
    y_d = nc.dram_tensor("y", [TT, 128, C], F16, kind="ExternalOutput").ap()

    with TileContext(nc) as tc:
        with (
            tc.tile_pool(name="persist", bufs=1) as pp,
        ):
            masks_sb = pp.tile([128, 2, T], F32, name="masks_sb")
            bqk_sb = pp.tile([128, 2 * NPAIR], F32, name="bqk_sb")
            bvb_sb = pp.tile([128, C], F16, name="bvb_sb")
            qkT = [pp.tile([128, T], F16, name=f"qkT{m}") for m in range(2 * NPAIR)]
            v_sb = [pp.tile([128, C], F16, name=f"v{t}") for t in range(TT)]
            # att.T tiles, fp16, dim1 = head parity; two sets alternating
            # per pair so pair a+1's QK overlaps pair a's AV
            att2 = [[pp.tile([128, 2, T], F16, name=f"att{s}_{t}")
                     for t in range(TT)] for s in range(2)]

            # ---------- Phase 1: QKV projection (fp8 DoubleRow, 3 digit
            # products into one PSUM group) ----------
            from contextlib import ExitStack
            with ExitStack() as stack:
                iop = stack.enter_context(tc.tile_pool(name="io", bufs=1))
                # 3 double-bank tiles (12KB/partition) shared by projection
                # windows and phase-2 merged QK pieces; + 2 single banks for
                # warmup / window-borrow / early-QK / AV
                ps_proj = stack.enter_context(
                    tc.tile_pool(name="psum_proj", bufs=3, space="PSUM"))
                ps_y = stack.enter_context(
                    tc.tile_pool(name="psum_y", bufs=2, space="PSUM"))
                yop = stack.enter_context(tc.tile_pool(name="yout", bufs=4))
                xh_sb = iop.tile([128, KT, T], F8, name="xh_sb")
                xl_sb = iop.tile([128, KT, T], F8, name="xl_sb")
                wv_h = iop.tile([128, KT, C], F8, name="wv_h")
                wv_l = iop.tile([128, KT, C], F8, name="wv_l")
                wq_h = iop.tile([128, KT, 2 * C], F8, name="wq_h")
                wq_l = iop.tile([128, KT, 2 * C], F8, name="wq_l")

                # PE p-state warmup on a zeroed scratch tile; results are
                # never read
                scratch = iop.tile([128, 512], F16, name="warm_src")
                nc.vector.memset(scratch[:], 0.0)
                warm = ps_y.tile([128, 512], F32, name="ps_warm", tag="ps_y")
                for w in WARM:
                    nc.tensor.matmul(warm[:, :w], scratch[:, :128],
                                     scratch[:, :w], start=True, stop=True)

                # input DMAs. The first windows' evictions gate on ALL three
                # contraction pairs, so the front ships x in column halves
                # and splits across the SP HWDGE queue and the Pool SWDGE
                # queue (parallel descriptor generators).
                sp, pool = nc.sync, nc.gpsimd
                sp.dma_start(out=wv_h[:, 0:2, :], in_=wvh[:, 0:2, :])
                sp.dma_start(out=xh_sb[:, 0:2, 0:256], in_=xh[:, 0:2, 0:256])
                pool.dma_start(out=xl_sb[:, 0:2, 0:512], in_=xl[:, 0:2, 0:512])
                sp.dma_start(out=wv_l[:, 0:2, :], in_=wvl[:, 0:2, :])
                sp.dma_start(out=xh_sb[:, 0:2, 256:512], in_=xh[:, 0:2, 256:512])
                pool.dma_start(out=xh_sb[:, 2:4, 0:512], in_=xh[:, 2:4, 0:512])
                sp.dma_start(out=wv_h[:, 2:4, :], in_=wvh[:, 2:4, :])
                pool.dma_start(out=xl_sb[:, 2:4, 0:512], in_=xl[:, 2:4, 0:512])
                sp.dma_start(out=wv_l[:, 2:4, :], in_=wvl[:, 2:4, :])
                pool.dma_start(out=xh_sb[:, 4:6, 0:512], in_=xh[:, 4:6, 0:512])
                sp.dma_start(out=wv_h[:, 4:6, :], in_=wvh[:, 4:6, :])
                pool.dma_start(out=xl_sb[:, 4:6, 0:512], in_=xl[:, 4:6, 0:512])
                sp.dma_start(out=wv_l[:, 4:6, :], in_=wvl[:, 4:6, :])
                sp.dma_start(out=bvb_sb[:], in_=bvb[:])
                # x column rests (v tiles 4-7 of the seq dim; windows 2-3)
                for p in range(NP):
                    pr = slice(2 * p, 2 * p + 2)
                    sp.dma_start(out=xh_sb[:, pr, 512:T], in_=xh[:, pr, 512:T])
                    pool.dma_start(out=xl_sb[:, pr, 512:T], in_=xl[:, pr, 512:T])
                # q/k weights; pair 0 in m0-m3 / m4-m11 halves
                pr0 = slice(0, 2)
                sp.dma_start(out=wq_h[:, pr0, :512], in_=wqh[:, pr0, :512])
                pool.dma_start(out=wq_l[:, pr0, :512], in_=wql[:, pr0, :512])
                sp.dma_start(out=wq_h[:, pr0, 512:], in_=wqh[:, pr0, 512:])
                pool.dma_start(out=wq_l[:, pr0, 512:], in_=wql[:, pr0, 512:])
                sp.dma_start(out=bqk_sb[:], in_=bqk[:])
                for p in range(1, NP):
                    prp = slice(2 * p, 2 * p + 2)
                    sp.dma_start(out=wq_h[:, prp, :], in_=wqh[:, prp, :])
                    pool.dma_start(out=wq_l[:, prp, :], in_=wql[:, prp, :])
                sp.dma_start(out=masks_sb[:], in_=masks[:])

                # each group = one [128, 512] PSUM bank lane.
                # ("v", t, n0, width) / ("qk", m, q0, width)
                groups = []
                for t in range(TT):
                    groups.append(("v", t, 0, 512))
                    groups.append(("v", t, 512, 256))
                for m in range(2 * NPAIR):
                    for q0 in (0, 512):
                        groups.append(("qk", m, q0, 512))

                # windows of 4 groups; k-pair-major, digit-product-minor so
                # PE consumption order matches DMA arrival order. Mid-stream
                # windows borrow the two ps_y banks for deeper pipelining.
                nwin = (len(groups) + 3) // 4
                for wi, w0 in enumerate(range(0, len(groups), 4)):
                    window = groups[w0:w0 + 4]
                    borrow = 3 <= wi < nwin - 2
                    if borrow:
                        dbl = ps_proj.tile([128, 2, 512], F32,
                                           name="ps_proj", tag="ps_proj")
                        tiles = [dbl[:, 0, :], dbl[:, 1, :],
                                 ps_y.tile([128, 512], F32, name="ps_b",
                                           tag="ps_y"),
                                 ps_y.tile([128, 512], F32, name="ps_b",
                                           tag="ps_y")][:len(window)]
                    else:
                        dbl = [ps_proj.tile([128, 2, 512], F32,
                                            name="ps_proj", tag="ps_proj")
                               for _ in range((len(window) + 1) // 2)]
                        tiles = [dbl[gi // 2][:, gi % 2, :]
                                 for gi in range(len(window))]
                    nmm = [0] * len(window)
                    total = [9 * (g[3] // 256) for g in window]
                    for p in range(NP):
                        pr = slice(2 * p, 2 * p + 2)
                        for term in range(3):
                            xa = (xh_sb, xl_sb, xh_sb)[term]
                            wva = (wv_h, wv_h, wv_l)[term]
                            wqa = (wq_h, wq_h, wq_l)[term]
                            for gi, (g, ps) in enumerate(zip(window, tiles)):
                                kind, i, o0, wd = g
                                for c0 in range(0, wd, 256):
                                    n = nmm[gi]
                                    nmm[gi] = n + 1
                                    st = n == 0
                                    sp_ = n == total[gi] - 1
                                    if kind == "v":
                                        nc.tensor.matmul(
                                            ps[:, c0:c0 + 256],
                                            xa[:, pr, 128 * i:128 * (i + 1)],
                                            wva[:, pr, o0 + c0:o0 + c0 + 256],
                                            start=st, stop=sp_, perf_mode=DR,
                                        )
                                    else:
                                        nc.tensor.matmul(
                                            ps[:, c0:c0 + 256],
                                            wqa[:, pr, 128 * i:128 * (i + 1)],
                                            xa[:, pr, o0 + c0:o0 + c0 + 256],
                                            start=st, stop=sp_, perf_mode=DR,
                                        )
                            if wi < 2 and term == 2:
                                # ramp-guard warmups at the early stall
                                # points (pair p+1 still in flight)
                                for _ in range(NSPRINKLE):
                                    nc.tensor.matmul(
                                        warm[:, :256], scratch[:, :128],
                                        scratch[:, :256], start=True,
                                        stop=True)
                    for g, ps in zip(window, tiles):
                        kind, i, o0, wd = g
                        if kind == "v":
                            nc.vector.scalar_tensor_tensor(
                                v_sb[i][:, o0:o0 + wd], ps[:, :wd], DESCALE,
                                bvb_sb[:, o0:o0 + wd],
                                AluOpType.mult, AluOpType.add,
                            )
                        elif i % 2 == 0:
                            nc.scalar.activation(
                                qkT[i][:, o0:o0 + wd], ps[:, :wd],
                                AF.Identity, bias=bqk_sb[:, i:i + 1],
                                scale=DESCALE,
                            )
                        else:
                            nc.vector.tensor_scalar(
                                qkT[i][:, o0:o0 + wd], ps[:, :wd],
                                DESCALE, bqk_sb[:, i:i + 1],
                                AluOpType.mult, AluOpType.add,
                            )

            # ---------- Phase 2: attention, pair by pair (same pool
            # scope: no phase barrier) ----------
                def qk_piece(att, qa, ka, tk, lo, hi, engine):
                    """One pair-merged QK piece covering q cols [lo, hi);
                    engine: 'dve' masked relu / 'act' pure relu / 'mix'
                    masked first 128 cols on DVE + relu rest on ACT."""
                    k0 = 128 * tk
                    ps = ps_proj.tile([128, 2, 512], F32, name="ps_qk",
                                      tag="ps_proj")
                    for r in range(2):
                        nc.tensor.matmul(
                            ps[:, r, :hi - lo],
                            ka[64 * r:64 * (r + 1), k0:k0 + 128],
                            qa[64 * r:64 * (r + 1), lo:hi],
                            start=True, stop=True,
                        )
                    if engine == "act":
                        nc.scalar.activation(att[tk][:, :, lo:hi],
                                             ps[:, :, :hi - lo], AF.Relu)
                    elif engine == "dve":
                        nc.vector.scalar_tensor_tensor(
                            att[tk][:, :, lo:hi], ps[:, :, :hi - lo],
                            0.0, masks_sb[:, :, :hi - lo],
                            AluOpType.max, AluOpType.mult,
                        )
                    else:  # mix: diag block on DVE, remainder on ACT
                        nc.vector.scalar_tensor_tensor(
                            att[tk][:, :, lo:lo + 128], ps[:, :, 0:128],
                            0.0, masks_sb[:, :, :128],
                            AluOpType.max, AluOpType.mult,
                        )
                        nc.scalar.activation(att[tk][:, :, lo + 128:hi],
                                             ps[:, :, 128:hi - lo], AF.Relu)

                def av_chunks(a):
                    # AV emission chunks for pair a: one closure per
                    # (bank, q-tile) group plus the bank evictions/DMAs
                    att = att2[a % 2]
                    state = {}

                    def group(bk, t):
                        def emit():
                            if bk not in state:
                                state[bk] = (
                                    ps_y.tile([128, 512], F32, name="ps_av",
                                              tag="ps_y"),
                                    yop.tile([128, 512], F16, name="y_sb",
                                             tag="y_sb"))
                            ps2, _ = state[bk]
                            for r in range(2):
                                h = 2 * a + r
                                col = 128 * (t % 4) + 64 * r
                                for k in range(t + 1):
                                    nc.tensor.matmul(
                                        ps2[:, col:col + 64],
                                        att[k][:, r, 128 * t:128 * (t + 1)],
                                        v_sb[k][:, 64 * h:64 * (h + 1)],
                                        start=(k == 0), stop=(k == t),
                                    )
                        return emit

                    def evict(bk):
                        def emit():
                            ps2, y_sb = state[bk]
                            if bk == 1:
                                nc.scalar.copy(y_sb[:], ps2[:])
                                nc.sync.dma_start(
                                    out=y_d[4:8, :, 128 * a:128 * (a + 1)]
                                        .transpose([1, 0, 2]),
                                    in_=y_sb[:])
                            else:
                                nc.vector.tensor_scalar(
                                    y_sb[:, 0:256], ps2[:, 0:256], 0.0, None,
                                    AluOpType.add)
                                nc.sync.dma_start(
                                    out=y_d[0:2, :, 128 * a:128 * (a + 1)]
                                        .transpose([1, 0, 2]),
                                    in_=y_sb[:, 0:256])
                                nc.scalar.copy(y_sb[:, 256:512],
                                               ps2[:, 256:512])
                                nc.sync.dma_start(
                                    out=y_d[2:4, :, 128 * a:128 * (a + 1)]
                                        .transpose([1, 0, 2]),
                                    in_=y_sb[:, 256:512])
                        return emit

                    return ([group(1, t) for t in range(4, 8)] + [evict(1)]
                            + [group(0, t) for t in range(4)] + [evict(0)])

                pending = []   # AV chunks of the previous pair
                for a in range(NPAIR):
                    att = att2[a % 2]
                    qa, ka = qkT[2 * a], qkT[2 * a + 1]
                    if a == 0:
                        # tk0/tk1 as unmerged singles through the spare ps_y
                        # ring: starts while the last projection windows
                        # still hold ps_proj
                        for tk in range(2):
                            k0 = 128 * tk
                            for r in range(2):
                                kh = ka[64 * r:64 * (r + 1), :]
                                qh = qa[64 * r:64 * (r + 1), :]
                                ps = ps_y.tile([128, 512], F32,
                                               name="ps_qk0", tag="ps_y")
                                nc.tensor.matmul(
                                    ps[:, k0:512], kh[:, k0:k0 + 128],
                                    qh[:, k0:512], start=True, stop=True,
                                )
                                nc.vector.scalar_tensor_tensor(
                                    att[tk][:, r, k0:512], ps[:, k0:512],
                                    0.0, masks_sb[:, 0, :512 - k0],
                                    AluOpType.max, AluOpType.mult,
                                )
                                ps = ps_y.tile([128, 512], F32,
                                               name="ps_qk0", tag="ps_y")
                                nc.tensor.matmul(
                                    ps[:], kh[:, k0:k0 + 128], qh[:, 512:T],
                                    start=True, stop=True,
                                )
                                nc.scalar.activation(
                                    att[tk][:, r, 512:T], ps[:], AF.Relu)
                    # piece order alternates DVE-evicted (masked) and
                    # ACT-evicted (pure-relu) pieces (the 3-slot psum ring
                    # recycles FIFO; same-engine runs convoy). The previous
                    # pair's AV chunks interleave into the emission so the
                    # PE always has filler during slot waits and the
                    # eviction engines never drain.
                    if a == 0:
                        order = [(5, 640, T, "dve"), (2, 512, T, "act"),
                                 (6, 768, T, "dve"), (3, 512, T, "act"),
                                 (7, 896, T, "dve"), (2, 256, 512, "dve"),
                                 (4, 512, T, "mix"), (3, 384, 512, "dve")]
                    else:
                        order = [(5, 640, T, "dve"), (0, 512, T, "act"),
                                 (6, 768, T, "dve"), (1, 512, T, "act"),
                                 (7, 896, T, "dve"), (2, 512, T, "act"),
                                 (0, 0, 512, "dve"), (3, 512, T, "act"),
                                 (1, 128, 512, "dve"), (4, 512, T, "mix"),
                                 (2, 256, 512, "dve"), (3, 384, 512, "dve")]
                    for tk, lo, hi, eng in order:
                        qk_piece(att, qa, ka, tk, lo, hi, eng)
                        if pending:
                            pending.pop(0)()
                    while pending:
                        pending.pop(0)()
                    pending = av_chunks(a)
                # drain the final pair's AV
                while pending:
                    pending.pop(0)()

    nc.compile()
    return nc


def _prep_host(x, W_attn, b_attn):
    s = 1.0 / np.sqrt(np.float32(HD))
    W = np.asarray(W_attn, dtype=np.float32).copy()
    b = np.asarray(b_attn, dtype=np.float32).copy()
    W[:C] *= s
    b[:C] *= s
    # interleave q/k head pairs: [q-pair0, k-pair0, q-pair1, k-pair1, ...], v natural
    rows = []
    for a in range(NPAIR):
        rows.extend(range(128 * a, 128 * (a + 1)))          # q heads 2a, 2a+1
        rows.extend(range(C + 128 * a, C + 128 * (a + 1)))  # k heads 2a, 2a+1
    rows.extend(range(2 * C, 3 * C))                        # v natural
    W_perm = W[rows]
    b_perm = b[rows]

    e4 = ml_dtypes.float8_e4m3

    def pack(mat):
        # (C, N) -> partition-major (128, KT, N): each partition's six
        # contraction k-tiles contiguous, k-pair-major
        Cr, N = mat.shape
        return np.ascontiguousarray(
            mat.reshape(KT, 128, N).transpose(1, 0, 2))

    def split8(mat):
        hi = mat.astype(e4)
        lo = (mat - hi.astype(np.float32)).astype(e4)
        return hi, lo

    wT = np.ascontiguousarray(W_perm.T) * np.float32(SW)     # (C, 3C)
    wqh, wql = split8(pack(wT[:, :2 * C]))
    wvh, wvl = split8(pack(wT[:, 2 * C:]))
    bqk = np.ascontiguousarray(b_perm[:2 * C].reshape(2 * NPAIR, 128).T)  # (128, 12)
    bvb = np.ascontiguousarray(
        np.broadcast_to(b_perm[2 * C:], (128, C))).astype(np.float16)
    tri = (np.arange(128)[None, :] >= np.arange(128)[:, None]).astype(np.float32)
    m1 = np.ones((128, T), dtype=np.float32)
    m1[:, 0:128] = tri             # kept windows always start at the diagonal
    masks = np.ascontiguousarray(
        np.broadcast_to(m1[:, None, :], (128, 2, T)))
    xT = np.asarray(x, dtype=np.float32).transpose(0, 2, 1) * np.float32(SX)  # (B, C, T)
    xhv = np.stack([pack(xT[c]) for c in range(B)])
    xhv, xlv = split8(xhv)
    return xhv, xlv, wqh, wql, wvh, wvl, bqk, bvb, masks


def kernel(x, W_attn, b_attn):
    if "nc" not in _CACHE:
        _CACHE["nc"] = _build()
    nc = _CACHE["nc"]

    (xhv, xlv, wqh, wql, wvh, wvl, bqk, bvb, masks) = _prep_host(x, W_attn, b_attn)
    in_maps = [
        {"xh": xhv[c], "xl": xlv[c], "wqh": wqh, "wql": wql, "wvh": wvh,
         "wvl": wvl, "bqk": bqk, "bvb": bvb, "masks": masks}
        for c in range(B)
    ]
    res = run_bass_kernel_spmd(nc, in_maps, list(range(B)))
    y = np.empty((B, T, C), dtype=np.float32)
    for c in range(B):
        y[c] = res.results[c]["y"].reshape(T, C).astype(np.float32)
    return y


# revision 7
# speedup vs baseline: 1.1017x; 1.0016x over previous
"""Trainium2 Bass kernel for causal masked-ReLU attention (no softmax).

Reference computation (B=8, T=1024, C=768, n_head=12, hd=64):
    qkv = x @ W_attn.T + b_attn
    q, k, v = split(qkv); per-head: att = relu(mask_causal(q k^T / sqrt(hd)))
    y = att @ v, heads re-merged -> (B, T, C)

Sharding: one batch element per NeuronCore (8 cores). Each core computes the
QKV projection and all 12 heads' attention for its batch element.

Layout strategy (per core):
  - Host passes x[b].T (C, T) and W.T (C, 3C) so the contraction dim C lands
    on SBUF partitions with unit-stride DMA (no on-chip transposes).
  - W rows are pre-permuted on host into [q-pair0, k-pair0, q-pair1, ...] so
    q.T / k.T of head h live at the same partition offset (h%2)*64 of their
    M-tiles; q weights/bias pre-scaled by 1/sqrt(hd).
  - QKV projection runs in fp8 (e4m3) DoubleRow perf mode: 256-deep
    contraction per pass at 0.5 cycles/row. Operands split into hi+lo fp8
    digits and three digit products xh*wh + xl*wh + xh*wl accumulate in one
    fp32 PSUM group (dropped xl*wl ~1e-4 relative); the 2^13 operand scale
    comes out at eviction with the bias. PE cost 0.75x of fp16.
  - att is computed transposed (att.T = k @ q.T, layout [T_k, T_q]), fp16
    everywhere, BOTH heads of a pair sharing one [128, 2, 512] double-bank
    PSUM tile and one [128, 2, T] SBUF tile per k-tile: the per-instruction
    PSUM-access penalty on DVE/ACT (~125/185ns) dominates phase 2, and
    pair-merging halves the eviction op count.
  - att tiles are double-buffered ACROSS pairs (sets alternate a%2): pair
    a+1's QK pieces stream through DVE/ACT while pair a's AV still reads
    the other set, so the mask-bound DVE runs and relu-bound ACT runs
    interleave instead of convoying (Pool cannot read PSUM, so evictions
    have only these two engines).
  - Per-pair piece order tk4..tk7, then [512:T) halves of tk0-3, then
    diagonal halves of tk0-3: AV bank-high's dependencies resolve
    mid-pair and bank-low's at the end, so the closing chain after the
    final AV matmul is one [128, 256] copy + one small DMA.
  - The AV product exploits weight-stationary asymmetry: per (q-tile,
    k-tile, head) matmul the STATIONARY operand is the [128, 128] att.T
    block and the MOVING operand is the head's 64 v columns (Ldweights is
    free; cost follows moving columns). Both heads accumulate into one
    [128, 512] PSUM bank (cols = 128*(t%4) + 64*head_parity + d); y
    DMAs out with a 3-d strided AP into natural (T, C) layout.
  - Causal masking at eviction via a [tri(128) | ones] relu-mask; DVE owns
    mask-needing pieces (only PSUM-capable engine with
    scalar_tensor_tensor), ACT the pure-relu halves; tile 4 splits
    [masked 2x128 | relu 2x384] to converge both at ~6.0us/pair, just
    above PE's 5.8us/pair.
  - The input front is split fine (x column halves) and spread across the
    SP HWDGE queue and the Pool SWDGE queue (which bypasses the shared
    HWDGE's ~625ns/DMA): the first window's eviction gates on all three
    contraction pairs, and a single queue starves the PE ~3us.
  - Warmup matmuls on a zeroed scratch tile ramp the PE p-state during the
    initial DMA wait, with more sprinkled into the first windows' stall
    points (idle gaps reset the ~3us ramp).
  - Projection windows: 2 groups per double-bank tile, 3 tiles rotating;
    mid-stream windows borrow the two spare ps_y banks for 8 groups in
    flight. One pool scope spans both phases (pool close = all-engine
    barrier); pair 0's first two k-tiles run unmerged through ps_y so
    phase 2 overlaps the last projection windows.
  - Output is written as y (T, C) in fp16; host upcasts.
"""

import numpy as np

import sys
for _p in ("/opt/trn_rl_repo", "/root/.axon_site", "/root/.axon_site/_ro/trn_rl_repo",
           "/root/.axon_site/_ro/pypackages"):
    if _p not in sys.path:
        sys.path.append(_p)

import ml_dtypes

import concourse.bacc as bacc
import concourse.mybir as mybir
from concourse.alu_op_type import AluOpType
from concourse.tile import TileContext
from concourse.bass_utils import run_bass_kernel_spmd

B, T, C = 8, 1024, 768
NH, HD = 12, 64
C3 = 3 * C            # 2304
KT = C // 128         # 6  contraction tiles of the projection
NP = KT // 2          # 3  contraction pairs (DoubleRow)
TT = T // 128         # 8  tiles of the sequence dim
NPAIR = NH // 2       # 6  head pairs
F32 = mybir.dt.float32
F16 = mybir.dt.float16
F8 = mybir.dt.float8e4
AF = mybir.ActivationFunctionType
DR = mybir.MatmulPerfMode.DoubleRow

SX = 16.0             # x pre-scale (keeps x-lo digits in e4m3 normal range)
SW = 512.0            # W pre-scale
DESCALE = 1.0 / (SX * SW)

# warmup matmul moving-widths (fp16): ramp the PE p-state during the
# initial input-DMA wait so real matmuls start at full clock
WARM = [512] * 4 + [256]
NSPRINKLE = 4         # 256-wide ramp-guard warmups per early stall point

_CACHE = {}


def _build():
    nc = bacc.Bacc("TRN2", target_bir_lowering=False, debug=False, num_devices=8)

    xh = nc.dram_tensor("xh", [128, KT, T], F8, kind="ExternalInput").ap()
    xl = nc.dram_tensor("xl", [128, KT, T], F8, kind="ExternalInput").ap()
    wvh = nc.dram_tensor("wvh", [128, KT, C], F8, kind="ExternalInput").ap()
    wvl = nc.dram_tensor("wvl", [128, KT, C], F8, kind="ExternalInput").ap()
    wqh = nc.dram_tensor("wqh", [128, KT, 2 * C], F8, kind="ExternalInput").ap()
    wql = nc.dram_tensor("wql", [128, KT, 2 * C], F8, kind="ExternalInput").ap()
    bqk = nc.dram_tensor("bqk", [128, 2 * NPAIR], F32, kind="ExternalInput").ap()
    bvb = nc.dram_tensor("bvb", [128, C], F16, kind="ExternalInput").ap()
    # masks = [tri(128) | ones(896)] duplicated along dim1 so pair-merged
    # [128, 2, W] evictions read the same relu-mask for both heads
    masks = nc.dram_tensor("masks", [128, 2, T], F32, kind="ExternalInput").ap()
    # y in natural (T, C) layout, tiled (TT, 128, C) for the 3-d AV DMAs
    y_d = nc.dram_tensor("y", [TT, 128, C], F16, kind="ExternalOutput").ap()

    with TileContext(nc) as tc:
        with (
            tc.tile_pool(name="persist", bufs=1) as pp,
        ):
            masks_sb = pp.tile([128, 2, T], F32, name="masks_sb")
            bqk_sb = pp.tile([128, 2 * NPAIR], F32, name="bqk_sb")
            bvb_sb = pp.tile([128, C], F16, name="bvb_sb")
            qkT = [pp.tile([128, T], F16, name=f"qkT{m}") for m in range(2 * NPAIR)]
            v_sb = [pp.tile([128, C], F16, name=f"v{t}") for t in range(TT)]
            # att.T tiles, fp16, dim1 = head parity; two sets alternating
            # per pair so pair a+1's QK overlaps pair a's AV
            att2 = [[pp.tile([128, 2, T], F16, name=f"att{s}_{t}")
                     for t in range(TT)] for s in range(2)]

            # ---------- Phase 1: QKV projection (fp8 DoubleRow, 3 digit
            # products into one PSUM group) ----------
            from contextlib import ExitStack
            with ExitStack() as stack:
                iop = stack.enter_context(tc.tile_pool(name="io", bufs=1))
                # ONE psum pool of 4 double-bank tiles (16KB/partition):
                # projection windows and phase-2 QK pieces rotate through
                # it, and each pair's two AV banks share one double. A
                # deeper ring matters more than reserved banks: the QK
                # piece rate is slot-latency-bound (Little's law).
                ps_proj = stack.enter_context(
                    tc.tile_pool(name="psum_all", bufs=4, space="PSUM"))
                yop = stack.enter_context(tc.tile_pool(name="yout", bufs=4))
                xh_sb = iop.tile([128, KT, T], F8, name="xh_sb")
                xl_sb = iop.tile([128, KT, T], F8, name="xl_sb")
                wv_h = iop.tile([128, KT, C], F8, name="wv_h")
                wv_l = iop.tile([128, KT, C], F8, name="wv_l")
                wq_h = iop.tile([128, KT, 2 * C], F8, name="wq_h")
                wq_l = iop.tile([128, KT, 2 * C], F8, name="wq_l")

                # PE p-state warmup on a zeroed scratch tile; results are
                # never read
                scratch = iop.tile([128, 512], F16, name="warm_src")
                nc.vector.memset(scratch[:], 0.0)
                warm2 = ps_proj.tile([128, 2, 512], F32, name="ps_warm",
                                     tag="ps_proj")
                warm = warm2[:, 0, :]
                for w in WARM:
                    nc.tensor.matmul(warm[:, :w], scratch[:, :128],
                                     scratch[:, :w], start=True, stop=True)

                # input DMAs. The first windows' evictions gate on ALL three
                # contraction pairs, so the front ships x in column halves
                # and splits across the SP HWDGE queue and the Pool SWDGE
                # queue (parallel descriptor generators).
                sp, pool = nc.sync, nc.gpsimd
                sp.dma_start(out=wv_h[:, 0:2, :], in_=wvh[:, 0:2, :])
                sp.dma_start(out=xh_sb[:, 0:2, 0:256], in_=xh[:, 0:2, 0:256])
                pool.dma_start(out=xl_sb[:, 0:2, 0:512], in_=xl[:, 0:2, 0:512])
                sp.dma_start(out=wv_l[:, 0:2, :], in_=wvl[:, 0:2, :])
                sp.dma_start(out=xh_sb[:, 0:2, 256:512], in_=xh[:, 0:2, 256:512])
                pool.dma_start(out=xh_sb[:, 2:4, 0:512], in_=xh[:, 2:4, 0:512])
                sp.dma_start(out=wv_h[:, 2:4, :], in_=wvh[:, 2:4, :])
                pool.dma_start(out=xl_sb[:, 2:4, 0:512], in_=xl[:, 2:4, 0:512])
                sp.dma_start(out=wv_l[:, 2:4, :], in_=wvl[:, 2:4, :])
                pool.dma_start(out=xh_sb[:, 4:6, 0:512], in_=xh[:, 4:6, 0:512])
                sp.dma_start(out=wv_h[:, 4:6, :], in_=wvh[:, 4:6, :])
                pool.dma_start(out=xl_sb[:, 4:6, 0:512], in_=xl[:, 4:6, 0:512])
                sp.dma_start(out=wv_l[:, 4:6, :], in_=wvl[:, 4:6, :])
                sp.dma_start(out=bvb_sb[:], in_=bvb[:])
                # x column rests (v tiles 4-7 of the seq dim; windows 2-3)
                for p in range(NP):
                    pr = slice(2 * p, 2 * p + 2)
                    sp.dma_start(out=xh_sb[:, pr, 512:T], in_=xh[:, pr, 512:T])
                    pool.dma_start(out=xl_sb[:, pr, 512:T], in_=xl[:, pr, 512:T])
                # q/k weights; pair 0 in m0-m3 / m4-m11 halves
                pr0 = slice(0, 2)
                sp.dma_start(out=wq_h[:, pr0, :512], in_=wqh[:, pr0, :512])
                pool.dma_start(out=wq_l[:, pr0, :512], in_=wql[:, pr0, :512])
                sp.dma_start(out=wq_h[:, pr0, 512:], in_=wqh[:, pr0, 512:])
                pool.dma_start(out=wq_l[:, pr0, 512:], in_=wql[:, pr0, 512:])
                sp.dma_start(out=bqk_sb[:], in_=bqk[:])
                for p in range(1, NP):
                    prp = slice(2 * p, 2 * p + 2)
                    sp.dma_start(out=wq_h[:, prp, :], in_=wqh[:, prp, :])
                    pool.dma_start(out=wq_l[:, prp, :], in_=wql[:, prp, :])
                sp.dma_start(out=masks_sb[:], in_=masks[:])

                # each group = one [128, 512] PSUM bank lane.
                # ("v", t, n0, width) / ("qk", m, q0, width)
                groups = []
                for t in range(TT):
                    groups.append(("v", t, 0, 512))
                    groups.append(("v", t, 512, 256))
                for m in range(2 * NPAIR):
                    for q0 in (0, 512):
                        groups.append(("qk", m, q0, 512))

                # windows of 4 groups; k-pair-major, digit-product-minor so
                # PE consumption order matches DMA arrival order. Mid-stream
                # windows borrow the two ps_y banks for deeper pipelining.
                nwin = (len(groups) + 3) // 4
                for wi, w0 in enumerate(range(0, len(groups), 4)):
                    window = groups[w0:w0 + 4]
                    dbl = [ps_proj.tile([128, 2, 512], F32,
                                        name="ps_proj", tag="ps_proj")
                           for _ in range((len(window) + 1) // 2)]
                    tiles = [dbl[gi // 2][:, gi % 2, :]
                             for gi in range(len(window))]
                    nmm = [0] * len(window)
                    total = [9 * (g[3] // 256) for g in window]
                    for p in range(NP):
                        pr = slice(2 * p, 2 * p + 2)
                        for term in range(3):
                            xa = (xh_sb, xl_sb, xh_sb)[term]
                            wva = (wv_h, wv_h, wv_l)[term]
                            wqa = (wq_h, wq_h, wq_l)[term]
                            for gi, (g, ps) in enumerate(zip(window, tiles)):
                                kind, i, o0, wd = g
                                for c0 in range(0, wd, 256):
                                    n = nmm[gi]
                                    nmm[gi] = n + 1
                                    st = n == 0
                                    sp_ = n == total[gi] - 1
                                    if kind == "v":
                                        nc.tensor.matmul(
                                            ps[:, c0:c0 + 256],
                                            xa[:, pr, 128 * i:128 * (i + 1)],
                                            wva[:, pr, o0 + c0:o0 + c0 + 256],
                                            start=st, stop=sp_, perf_mode=DR,
                                        )
                                    else:
                                        nc.tensor.matmul(
                                            ps[:, c0:c0 + 256],
                                            wqa[:, pr, 128 * i:128 * (i + 1)],
                                            xa[:, pr, o0 + c0:o0 + c0 + 256],
                                            start=st, stop=sp_, perf_mode=DR,
                                        )
                            if wi < 2 and term == 2:
                                # ramp-guard warmups at the early stall
                                # points (pair p+1 still in flight)
                                for _ in range(NSPRINKLE):
                                    nc.tensor.matmul(
                                        warm[:, :256], scratch[:, :128],
                                        scratch[:, :256], start=True,
                                        stop=True)
                    for g, ps in zip(window, tiles):
                        kind, i, o0, wd = g
                        if kind == "v":
                            nc.vector.scalar_tensor_tensor(
                                v_sb[i][:, o0:o0 + wd], ps[:, :wd], DESCALE,
                                bvb_sb[:, o0:o0 + wd],
                                AluOpType.mult, AluOpType.add,
                            )
                        elif i % 2 == 0:
                            nc.scalar.activation(
                                qkT[i][:, o0:o0 + wd], ps[:, :wd],
                                AF.Identity, bias=bqk_sb[:, i:i + 1],
                                scale=DESCALE,
                            )
                        else:
                            nc.vector.tensor_scalar(
                                qkT[i][:, o0:o0 + wd], ps[:, :wd],
                                DESCALE, bqk_sb[:, i:i + 1],
                                AluOpType.mult, AluOpType.add,
                            )

            # ---------- Phase 2: attention, pair by pair (same pool
            # scope: no phase barrier) ----------
                def qk_piece(att, qa, ka, tk, lo, hi, engine):
                    """One pair-merged QK piece covering q cols [lo, hi);
                    engine: 'dve' masked relu / 'act' pure relu / 'mix'
                    masked first 128 cols on DVE + relu rest on ACT."""
                    k0 = 128 * tk
                    ps = ps_proj.tile([128, 2, 512], F32, name="ps_qk",
                                      tag="ps_proj")
                    for r in range(2):
                        nc.tensor.matmul(
                            ps[:, r, :hi - lo],
                            ka[64 * r:64 * (r + 1), k0:k0 + 128],
                            qa[64 * r:64 * (r + 1), lo:hi],
                            start=True, stop=True,
                        )
                    if engine == "act":
                        nc.scalar.activation(att[tk][:, :, lo:hi],
                                             ps[:, :, :hi - lo], AF.Relu)
                    elif engine == "dve":
                        nc.vector.scalar_tensor_tensor(
                            att[tk][:, :, lo:hi], ps[:, :, :hi - lo],
                            0.0, masks_sb[:, :, :hi - lo],
                            AluOpType.max, AluOpType.mult,
                        )
                    else:  # mix: diag block on DVE, remainder on ACT
                        nc.vector.scalar_tensor_tensor(
                            att[tk][:, :, lo:lo + 128], ps[:, :, 0:128],
                            0.0, masks_sb[:, :, :128],
                            AluOpType.max, AluOpType.mult,
                        )
                        nc.scalar.activation(att[tk][:, :, lo + 128:hi],
                                             ps[:, :, 128:hi - lo], AF.Relu)

                def av_chunks(a):
                    # AV emission chunks for pair a: one closure per
                    # (bank, q-tile) group plus the bank evictions/DMAs.
                    # Both banks live in one double-bank psum tile
                    # (lane 0 = high bank, lane 1 = low bank).
                    att = att2[a % 2]
                    state = {}

                    def get():
                        if "ps" not in state:
                            state["ps"] = ps_proj.tile(
                                [128, 2, 512], F32, name="ps_av",
                                tag="ps_proj")
                        return state["ps"]

                    def group(bk, t):
                        def emit():
                            ps2 = get()[:, 1 - bk, :]
                            for r in range(2):
                                h = 2 * a + r
                                col = 128 * (t % 4) + 64 * r
                                for k in range(t + 1):
                                    nc.tensor.matmul(
                                        ps2[:, col:col + 64],
                                        att[k][:, r, 128 * t:128 * (t + 1)],
                                        v_sb[k][:, 64 * h:64 * (h + 1)],
                                        start=(k == 0), stop=(k == t),
                                    )
                        return emit

                    def evict(bk):
                        def emit():
                            ps2 = state["ps"][:, 1 - bk, :]
                            y_sb = yop.tile([128, 512], F16, name="y_sb",
                                            tag="y_sb")
                            if bk == 1:
                                nc.scalar.copy(y_sb[:], ps2[:])
                                nc.sync.dma_start(
                                    out=y_d[4:8, :, 128 * a:128 * (a + 1)]
                                        .transpose([1, 0, 2]),
                                    in_=y_sb[:])
                            else:
                                nc.vector.tensor_scalar(
                                    y_sb[:, 0:256], ps2[:, 0:256], 0.0, None,
                                    AluOpType.add)
                                nc.sync.dma_start(
                                    out=y_d[0:2, :, 128 * a:128 * (a + 1)]
                                        .transpose([1, 0, 2]),
                                    in_=y_sb[:, 0:256])
                                nc.scalar.copy(y_sb[:, 256:512],
                                               ps2[:, 256:512])
                                nc.sync.dma_start(
                                    out=y_d[2:4, :, 128 * a:128 * (a + 1)]
                                        .transpose([1, 0, 2]),
                                    in_=y_sb[:, 256:512])
                        return emit

                    return ([group(1, t) for t in range(4, 8)] + [evict(1)]
                            + [group(0, t) for t in range(4)] + [evict(0)])

                pending = []   # AV chunks of the previous pair
                for a in range(NPAIR):
                    att = att2[a % 2]
                    qa, ka = qkT[2 * a], qkT[2 * a + 1]
                    # piece order alternates DVE-evicted (masked) and
                    # ACT-evicted (pure-relu) pieces (the psum ring recycles
                    # FIFO; same-engine runs convoy both engines). The
                    # previous pair's AV chunks interleave into the
                    # emission so the PE always has filler during slot
                    # waits and the eviction engines never drain.
                    order = [(5, 640, T, "dve"), (0, 512, T, "act"),
                             (6, 768, T, "dve"), (1, 512, T, "act"),
                             (7, 896, T, "dve"), (2, 512, T, "act"),
                             (0, 0, 512, "dve"), (3, 512, T, "act"),
                             (1, 128, 512, "dve"), (4, 512, T, "mix"),
                             (2, 256, 512, "dve"), (3, 384, 512, "dve")]
                    mine = av_chunks(a) if a == NPAIR - 1 else None
                    for pi, (tk, lo, hi, eng) in enumerate(order):
                        qk_piece(att, qa, ka, tk, lo, hi, eng)
                        if pending:
                            pending.pop(0)()
                        if mine is not None and pi >= 9:
                            # final pair: its bank-high groups interleave
                            # behind the pieces they depend on (all p2s and
                            # tk4-7 are evicted by piece 10)
                            mine.pop(0)()
                    while pending:
                        pending.pop(0)()
                    if mine is not None:
                        for c in mine:
                            c()
                    else:
                        pending = av_chunks(a)

    nc.compile()
    return nc


def _prep_host(x, W_attn, b_attn):
    s = 1.0 / np.sqrt(np.float32(HD))
    W = np.asarray(W_attn, dtype=np.float32).copy()
    b = np.asarray(b_attn, dtype=np.float32).copy()
    W[:C] *= s
    b[:C] *= s
    # interleave q/k head pairs: [q-pair0, k-pair0, q-pair1, k-pair1, ...], v natural
    rows = []
    for a in range(NPAIR):
        rows.extend(range(128 * a, 128 * (a + 1)))          # q heads 2a, 2a+1
        rows.extend(range(C + 128 * a, C + 128 * (a + 1)))  # k heads 2a, 2a+1
    rows.extend(range(2 * C, 3 * C))                        # v natural
    W_perm = W[rows]
    b_perm = b[rows]

    e4 = ml_dtypes.float8_e4m3

    def pack(mat):
        # (C, N) -> partition-major (128, KT, N): each partition's six
        # contraction k-tiles contiguous, k-pair-major
        Cr, N = mat.shape
        return np.ascontiguousarray(
            mat.reshape(KT, 128, N).transpose(1, 0, 2))

    def split8(mat):
        hi = mat.astype(e4)
        lo = (mat - hi.astype(np.float32)).astype(e4)
        return hi, lo

    wT = np.ascontiguousarray(W_perm.T) * np.float32(SW)     # (C, 3C)
    wqh, wql = split8(pack(wT[:, :2 * C]))
    wvh, wvl = split8(pack(wT[:, 2 * C:]))
    bqk = np.ascontiguousarray(b_perm[:2 * C].reshape(2 * NPAIR, 128).T)  # (128, 12)
    bvb = np.ascontiguousarray(
        np.broadcast_to(b_perm[2 * C:], (128, C))).astype(np.float16)
    tri = (np.arange(128)[None, :] >= np.arange(128)[:, None]).astype(np.float32)
    m1 = np.ones((128, T), dtype=np.float32)
    m1[:, 0:128] = tri             # kept windows always start at the diagonal
    masks = np.ascontiguousarray(
        np.broadcast_to(m1[:, None, :], (128, 2, T)))
    xT = np.asarray(x, dtype=np.float32).transpose(0, 2, 1) * np.float32(SX)  # (B, C, T)
    xhv = np.stack([pack(xT[c]) for c in range(B)])
    xhv, xlv = split8(xhv)
    return xhv, xlv, wqh, wql, wvh, wvl, bqk, bvb, masks


def kernel(x, W_attn, b_attn):
    if "nc" not in _CACHE:
        _CACHE["nc"] = _build()
    nc = _CACHE["nc"]

    (xhv, xlv, wqh, wql, wvh, wvl, bqk, bvb, masks) = _prep_host(x, W_attn, b_attn)
    in_maps = [
        {"xh": xhv[c], "xl": xlv[c], "wqh": wqh, "wql": wql, "wvh": wvh,
         "wvl": wvl, "bqk": bqk, "bvb": bvb, "masks": masks}
        for c in range(B)
    ]
    res = run_bass_kernel_spmd(nc, in_maps, list(range(B)))
    y = np.empty((B, T, C), dtype=np.float32)
    for c in range(B):
        y[c] = res.results[c]["y"].reshape(T, C).astype(np.float32)
    return y


# revision 8
# speedup vs baseline: 1.1064x; 1.0042x over previous
"""Trainium2 Bass kernel for causal masked-ReLU attention (no softmax).

Reference computation (B=8, T=1024, C=768, n_head=12, hd=64):
    qkv = x @ W_attn.T + b_attn
    q, k, v = split(qkv); per-head: att = relu(mask_causal(q k^T / sqrt(hd)))
    y = att @ v, heads re-merged -> (B, T, C)

Sharding: one batch element per NeuronCore (8 cores). Each core computes the
QKV projection and all 12 heads' attention for its batch element.

Layout strategy (per core):
  - Host passes x[b].T (C, T) and W.T (C, 3C) so the contraction dim C lands
    on SBUF partitions with unit-stride DMA (no on-chip transposes).
  - W rows are pre-permuted on host into [q-pair0, k-pair0, q-pair1, ...] so
    q.T / k.T of head h live at the same partition offset (h%2)*64 of their
    M-tiles; q weights/bias pre-scaled by 1/sqrt(hd).
  - QKV projection runs in fp8 (e4m3) DoubleRow perf mode: 256-deep
    contraction per pass at 0.5 cycles/row. Operands split into hi+lo fp8
    digits and three digit products xh*wh + xl*wh + xh*wl accumulate in one
    fp32 PSUM group (dropped xl*wl ~1e-4 relative); the 2^13 operand scale
    comes out at eviction with the bias. PE cost 0.75x of fp16.
  - att is computed transposed (att.T = k @ q.T, layout [T_k, T_q]), fp16
    everywhere, BOTH heads of a pair sharing one [128, 2, 512] double-bank
    PSUM tile and one [128, 2, T] SBUF tile per k-tile: the per-instruction
    PSUM-access penalty on DVE/ACT (~125/185ns) dominates phase 2, and
    pair-merging halves the eviction op count.
  - att tiles are double-buffered ACROSS pairs (sets alternate a%2): pair
    a+1's QK pieces stream through DVE/ACT while pair a's AV still reads
    the other set, so the mask-bound DVE runs and relu-bound ACT runs
    interleave instead of convoying (Pool cannot read PSUM, so evictions
    have only these two engines).
  - Per-pair piece order tk4..tk7, then [512:T) halves of tk0-3, then
    diagonal halves of tk0-3: AV bank-high's dependencies resolve
    mid-pair and bank-low's at the end, so the closing chain after the
    final AV matmul is one [128, 256] copy + one small DMA.
  - The AV product exploits weight-stationary asymmetry: per (q-tile,
    k-tile, head) matmul the STATIONARY operand is the [128, 128] att.T
    block and the MOVING operand is the head's 64 v columns (Ldweights is
    free; cost follows moving columns). Both heads accumulate into one
    [128, 512] PSUM bank (cols = 128*(t%4) + 64*head_parity + d); y
    DMAs out with a 3-d strided AP into natural (T, C) layout.
  - Causal masking at eviction via a [tri(128) | ones] relu-mask; DVE owns
    mask-needing pieces (only PSUM-capable engine with
    scalar_tensor_tensor), ACT the pure-relu halves; tile 4 splits
    [masked 2x128 | relu 2x384] to converge both at ~6.0us/pair, just
    above PE's 5.8us/pair.
  - The input front is split fine (x column halves) and spread across the
    SP HWDGE queue and the Pool SWDGE queue (which bypasses the shared
    HWDGE's ~625ns/DMA): the first window's eviction gates on all three
    contraction pairs, and a single queue starves the PE ~3us.
  - Warmup matmuls on a zeroed scratch tile ramp the PE p-state during the
    initial DMA wait, with more sprinkled into the first windows' stall
    points (idle gaps reset the ~3us ramp).
  - Projection windows: 2 groups per double-bank tile, 3 tiles rotating;
    mid-stream windows borrow the two spare ps_y banks for 8 groups in
    flight. One pool scope spans both phases (pool close = all-engine
    barrier); pair 0's first two k-tiles run unmerged through ps_y so
    phase 2 overlaps the last projection windows.
  - Output is written as y (T, C) in fp16; host upcasts.
"""

import numpy as np

import sys
for _p in ("/opt/trn_rl_repo", "/root/.axon_site", "/root/.axon_site/_ro/trn_rl_repo",
           "/root/.axon_site/_ro/pypackages"):
    if _p not in sys.path:
        sys.path.append(_p)

import ml_dtypes

import concourse.bacc as bacc
import concourse.mybir as mybir
from concourse.alu_op_type import AluOpType
from concourse.tile import TileContext
from concourse.bass_utils import run_bass_kernel_spmd

B, T, C = 8, 1024, 768
NH, HD = 12, 64
C3 = 3 * C            # 2304
KT = C // 128         # 6  contraction tiles of the projection
NP = KT // 2          # 3  contraction pairs (DoubleRow)
TT = T // 128         # 8  tiles of the sequence dim
NPAIR = NH // 2       # 6  head pairs
F32 = mybir.dt.float32
F16 = mybir.dt.float16
F8 = mybir.dt.float8e4
AF = mybir.ActivationFunctionType
DR = mybir.MatmulPerfMode.DoubleRow

SX = 16.0             # x pre-scale (keeps x-lo digits in e4m3 normal range)
SW = 512.0            # W pre-scale
DESCALE = 1.0 / (SX * SW)

# warmup matmul moving-widths (fp16): ramp the PE p-state during the
# initial input-DMA wait so real matmuls start at full clock
WARM = [512] * 4 + [256]
NSPRINKLE = 4         # 256-wide ramp-guard warmups per early stall point

_CACHE = {}


def _build():
    nc = bacc.Bacc("TRN2", target_bir_lowering=False, debug=False, num_devices=8)

    xh = nc.dram_tensor("xh", [128, KT, T], F8, kind="ExternalInput").ap()
    xl = nc.dram_tensor("xl", [128, KT, T], F8, kind="ExternalInput").ap()
    wvh = nc.dram_tensor("wvh", [128, KT, C], F8, kind="ExternalInput").ap()
    wvl = nc.dram_tensor("wvl", [128, KT, C], F8, kind="ExternalInput").ap()
    wqh = nc.dram_tensor("wqh", [128, KT, 2 * C], F8, kind="ExternalInput").ap()
    wql = nc.dram_tensor("wql", [128, KT, 2 * C], F8, kind="ExternalInput").ap()
    bqk = nc.dram_tensor("bqk", [128, 2 * NPAIR], F32, kind="ExternalInput").ap()
    bvb = nc.dram_tensor("bvb", [128, C], F16, kind="ExternalInput").ap()
    # masks = [tri(128) | ones(896)] duplicated along dim1 so pair-merged
    # [128, 2, W] evictions read the same relu-mask for both heads
    masks = nc.dram_tensor("masks", [128, 2, T], F32, kind="ExternalInput").ap()
    # y in natural (T, C) layout, tiled (TT, 128, C) for the 3-d AV DMAs
    y_d = nc.dram_tensor("y", [TT, 128, C], F16, kind="ExternalOutput").ap()

    with TileContext(nc) as tc:
        with (
            tc.tile_pool(name="persist", bufs=1) as pp,
        ):
            masks_sb = pp.tile([128, 2, T], F32, name="masks_sb")
            bqk_sb = pp.tile([128, 2 * NPAIR], F32, name="bqk_sb")
            bvb_sb = pp.tile([128, C], F16, name="bvb_sb")
            qkT = [pp.tile([128, T], F16, name=f"qkT{m}") for m in range(2 * NPAIR)]
            v_sb = [pp.tile([128, C], F16, name=f"v{t}") for t in range(TT)]
            # att.T tiles, fp16, dim1 = head parity; two sets alternating
            # per pair so pair a+1's QK overlaps pair a's AV
            att2 = [[pp.tile([128, 2, T], F16, name=f"att{s}_{t}")
                     for t in range(TT)] for s in range(2)]

            # ---------- Phase 1: QKV projection (fp8 DoubleRow, 3 digit
            # products into one PSUM group) ----------
            from contextlib import ExitStack
            with ExitStack() as stack:
                iop = stack.enter_context(tc.tile_pool(name="io", bufs=1))
                # 6 single-bank tiles for projection windows and phase-2
                # QK pieces (a DEEP ring: eviction backlog depth is what
                # lets DVE/ACT run at ~100%), + 2 single banks for
                # warmup / window-borrow / AV
                ps_proj = stack.enter_context(
                    tc.tile_pool(name="psum_qk", bufs=6, space="PSUM"))
                ps_av = stack.enter_context(
                    tc.tile_pool(name="psum_av", bufs=2, space="PSUM"))
                yop = stack.enter_context(tc.tile_pool(name="yout", bufs=4))
                xh_sb = iop.tile([128, KT, T], F8, name="xh_sb")
                xl_sb = iop.tile([128, KT, T], F8, name="xl_sb")
                wv_h = iop.tile([128, KT, C], F8, name="wv_h")
                wv_l = iop.tile([128, KT, C], F8, name="wv_l")
                wq_h = iop.tile([128, KT, 2 * C], F8, name="wq_h")
                wq_l = iop.tile([128, KT, 2 * C], F8, name="wq_l")

                # PE p-state warmup on a zeroed scratch tile; results are
                # never read
                scratch = iop.tile([128, 512], F16, name="warm_src")
                nc.vector.memset(scratch[:], 0.0)
                warm = ps_av.tile([128, 512], F32, name="ps_warm",
                                  tag="ps_av")
                for w in WARM:
                    nc.tensor.matmul(warm[:, :w], scratch[:, :128],
                                     scratch[:, :w], start=True, stop=True)

                # input DMAs. The first windows' evictions gate on ALL three
                # contraction pairs, so the front ships x in column halves
                # and splits across the SP HWDGE queue and the Pool SWDGE
                # queue (parallel descriptor generators).
                sp, pool = nc.sync, nc.gpsimd
                sp.dma_start(out=wv_h[:, 0:2, :], in_=wvh[:, 0:2, :])
                sp.dma_start(out=xh_sb[:, 0:2, 0:256], in_=xh[:, 0:2, 0:256])
                pool.dma_start(out=xl_sb[:, 0:2, 0:512], in_=xl[:, 0:2, 0:512])
                sp.dma_start(out=wv_l[:, 0:2, :], in_=wvl[:, 0:2, :])
                sp.dma_start(out=xh_sb[:, 0:2, 256:512], in_=xh[:, 0:2, 256:512])
                pool.dma_start(out=xh_sb[:, 2:4, 0:512], in_=xh[:, 2:4, 0:512])
                sp.dma_start(out=wv_h[:, 2:4, :], in_=wvh[:, 2:4, :])
                pool.dma_start(out=xl_sb[:, 2:4, 0:512], in_=xl[:, 2:4, 0:512])
                sp.dma_start(out=wv_l[:, 2:4, :], in_=wvl[:, 2:4, :])
                pool.dma_start(out=xh_sb[:, 4:6, 0:512], in_=xh[:, 4:6, 0:512])
                sp.dma_start(out=wv_h[:, 4:6, :], in_=wvh[:, 4:6, :])
                pool.dma_start(out=xl_sb[:, 4:6, 0:512], in_=xl[:, 4:6, 0:512])
                sp.dma_start(out=wv_l[:, 4:6, :], in_=wvl[:, 4:6, :])
                sp.dma_start(out=bvb_sb[:], in_=bvb[:])
                # x column rests (v tiles 4-7 of the seq dim; windows 2-3)
                for p in range(NP):
                    pr = slice(2 * p, 2 * p + 2)
                    sp.dma_start(out=xh_sb[:, pr, 512:T], in_=xh[:, pr, 512:T])
                    pool.dma_start(out=xl_sb[:, pr, 512:T], in_=xl[:, pr, 512:T])
                # q/k weights; pair 0 in m0-m3 / m4-m11 halves
                pr0 = slice(0, 2)
                sp.dma_start(out=wq_h[:, pr0, :512], in_=wqh[:, pr0, :512])
                pool.dma_start(out=wq_l[:, pr0, :512], in_=wql[:, pr0, :512])
                sp.dma_start(out=wq_h[:, pr0, 512:], in_=wqh[:, pr0, 512:])
                pool.dma_start(out=wq_l[:, pr0, 512:], in_=wql[:, pr0, 512:])
                sp.dma_start(out=bqk_sb[:], in_=bqk[:])
                for p in range(1, NP):
                    prp = slice(2 * p, 2 * p + 2)
                    sp.dma_start(out=wq_h[:, prp, :], in_=wqh[:, prp, :])
                    pool.dma_start(out=wq_l[:, prp, :], in_=wql[:, prp, :])
                sp.dma_start(out=masks_sb[:], in_=masks[:])

                # each group = one [128, 512] PSUM bank lane.
                # ("v", t, n0, width) / ("qk", m, q0, width)
                groups = []
                for t in range(TT):
                    groups.append(("v", t, 0, 512))
                    groups.append(("v", t, 512, 256))
                for m in range(2 * NPAIR):
                    for q0 in (0, 512):
                        groups.append(("qk", m, q0, 512))

                # windows of 4 groups; k-pair-major, digit-product-minor so
                # PE consumption order matches DMA arrival order. Mid-stream
                # windows borrow the two ps_y banks for deeper pipelining.
                nwin = (len(groups) + 3) // 4
                for wi, w0 in enumerate(range(0, len(groups), 4)):
                    window = groups[w0:w0 + 4]
                    borrow = 3 <= wi < nwin - 2
                    tiles = []
                    for gi in range(len(window)):
                        if borrow and gi >= 3:
                            tiles.append(ps_av.tile([128, 512], F32,
                                                    name="ps_b", tag="ps_av"))
                        else:
                            tiles.append(ps_proj.tile([128, 512], F32,
                                                      name="ps_w",
                                                      tag="ps_qk"))
                    nmm = [0] * len(window)
                    total = [9 * (g[3] // 256) for g in window]
                    for p in range(NP):
                        pr = slice(2 * p, 2 * p + 2)
                        for term in range(3):
                            xa = (xh_sb, xl_sb, xh_sb)[term]
                            wva = (wv_h, wv_h, wv_l)[term]
                            wqa = (wq_h, wq_h, wq_l)[term]
                            for gi, (g, ps) in enumerate(zip(window, tiles)):
                                kind, i, o0, wd = g
                                for c0 in range(0, wd, 256):
                                    n = nmm[gi]
                                    nmm[gi] = n + 1
                                    st = n == 0
                                    sp_ = n == total[gi] - 1
                                    if kind == "v":
                                        nc.tensor.matmul(
                                            ps[:, c0:c0 + 256],
                                            xa[:, pr, 128 * i:128 * (i + 1)],
                                            wva[:, pr, o0 + c0:o0 + c0 + 256],
                                            start=st, stop=sp_, perf_mode=DR,
                                        )
                                    else:
                                        nc.tensor.matmul(
                                            ps[:, c0:c0 + 256],
                                            wqa[:, pr, 128 * i:128 * (i + 1)],
                                            xa[:, pr, o0 + c0:o0 + c0 + 256],
                                            start=st, stop=sp_, perf_mode=DR,
                                        )
                            if wi < 2 and term == 2:
                                # ramp-guard warmups at the early stall
                                # points (pair p+1 still in flight)
                                for _ in range(NSPRINKLE):
                                    nc.tensor.matmul(
                                        warm[:, :256], scratch[:, :128],
                                        scratch[:, :256], start=True,
                                        stop=True)
                    for g, ps in zip(window, tiles):
                        kind, i, o0, wd = g
                        if kind == "v":
                            nc.vector.scalar_tensor_tensor(
                                v_sb[i][:, o0:o0 + wd], ps[:, :wd], DESCALE,
                                bvb_sb[:, o0:o0 + wd],
                                AluOpType.mult, AluOpType.add,
                            )
                        elif i % 2 == 0:
                            nc.scalar.activation(
                                qkT[i][:, o0:o0 + wd], ps[:, :wd],
                                AF.Identity, bias=bqk_sb[:, i:i + 1],
                                scale=DESCALE,
                            )
                        else:
                            nc.vector.tensor_scalar(
                                qkT[i][:, o0:o0 + wd], ps[:, :wd],
                                DESCALE, bqk_sb[:, i:i + 1],
                                AluOpType.mult, AluOpType.add,
                            )

            # ---------- Phase 2: attention, pair by pair (same pool
            # scope: no phase barrier) ----------
                def qk_piece(att, qa, ka, tk, r, lo, hi, engine):
                    """One per-head QK piece covering q cols [lo, hi);
                    engine: 'dve' masked relu / 'act' pure relu / 'mix'
                    masked first 128 cols on DVE + relu rest on ACT."""
                    k0 = 128 * tk
                    kh = ka[64 * r:64 * (r + 1), :]
                    qh = qa[64 * r:64 * (r + 1), :]
                    ps = ps_proj.tile([128, 512], F32, name="ps_qk",
                                      tag="ps_qk")
                    nc.tensor.matmul(
                        ps[:, :hi - lo], kh[:, k0:k0 + 128], qh[:, lo:hi],
                        start=True, stop=True,
                    )
                    if engine == "act":
                        nc.scalar.activation(att[tk][:, r, lo:hi],
                                             ps[:, :hi - lo], AF.Relu)
                    elif engine == "dve":
                        nc.vector.scalar_tensor_tensor(
                            att[tk][:, r, lo:hi], ps[:, :hi - lo],
                            0.0, masks_sb[:, 0, :hi - lo],
                            AluOpType.max, AluOpType.mult,
                        )
                    else:  # mix: diag block on DVE, remainder on ACT
                        nc.vector.scalar_tensor_tensor(
                            att[tk][:, r, lo:lo + 128], ps[:, 0:128],
                            0.0, masks_sb[:, 0, :128],
                            AluOpType.max, AluOpType.mult,
                        )
                        nc.scalar.activation(att[tk][:, r, lo + 128:hi],
                                             ps[:, 128:hi - lo], AF.Relu)

                def av_chunks(a):
                    # AV emission chunks for pair a: one closure per
                    # (bank, q-tile) group plus the bank evictions/DMAs;
                    # banks live in separate single-bank tiles so each
                    # frees as soon as its own eviction completes
                    att = att2[a % 2]
                    state = {}

                    def group(bk, t):
                        def emit():
                            if bk not in state:
                                state[bk] = ps_av.tile(
                                    [128, 512], F32, name="ps_av",
                                    tag="ps_av")
                            ps2 = state[bk]
                            for r in range(2):
                                h = 2 * a + r
                                col = 128 * (t % 4) + 64 * r
                                for k in range(t + 1):
                                    nc.tensor.matmul(
                                        ps2[:, col:col + 64],
                                        att[k][:, r, 128 * t:128 * (t + 1)],
                                        v_sb[k][:, 64 * h:64 * (h + 1)],
                                        start=(k == 0), stop=(k == t),
                                    )
                        return emit

                    def evict(bk):
                        def emit():
                            ps2 = state[bk]
                            y_sb = yop.tile([128, 512], F16, name="y_sb",
                                            tag="y_sb")
                            if bk == 1:
                                nc.vector.tensor_scalar(
                                    y_sb[:, 0:256], ps2[:, 0:256], 0.0,
                                    None, AluOpType.add)
                                nc.scalar.copy(y_sb[:, 256:512],
                                               ps2[:, 256:512])
                                nc.sync.dma_start(
                                    out=y_d[4:8, :, 128 * a:128 * (a + 1)]
                                        .transpose([1, 0, 2]),
                                    in_=y_sb[:])
                            else:
                                nc.scalar.copy(y_sb[:, 0:256], ps2[:, 0:256])
                                nc.sync.dma_start(
                                    out=y_d[0:2, :, 128 * a:128 * (a + 1)]
                                        .transpose([1, 0, 2]),
                                    in_=y_sb[:, 0:256])
                                nc.scalar.copy(y_sb[:, 256:512],
                                               ps2[:, 256:512])
                                nc.sync.dma_start(
                                    out=y_d[2:4, :, 128 * a:128 * (a + 1)]
                                        .transpose([1, 0, 2]),
                                    in_=y_sb[:, 256:512])
                        return emit

                    return ([group(1, t) for t in range(4, 8)] + [evict(1)]
                            + [group(0, t) for t in range(4)] + [evict(0)])

                # per-pair piece order: tk4 + the [512:T) halves early
                # (bank-high AV deps), diagonal halves of tk0-3 last
                # (bank-low deps), DVE/ACT interleaved where possible
                ORDER = [(4, 0, 512, T, "mix"), (4, 1, 512, T, "mix"),
                         (0, 0, 512, T, "act"), (5, 0, 640, T, "dve"),
                         (0, 1, 512, T, "act"), (5, 1, 640, T, "dve"),
                         (1, 0, 512, T, "act"), (6, 0, 768, T, "dve"),
                         (1, 1, 512, T, "act"), (6, 1, 768, T, "dve"),
                         (2, 0, 512, T, "act"), (7, 0, 896, T, "dve"),
                         (2, 1, 512, T, "act"), (7, 1, 896, T, "dve"),
                         (3, 0, 512, T, "act"), (0, 0, 0, 512, "dve"),
                         (3, 1, 512, T, "act"), (0, 1, 0, 512, "dve"),
                         (1, 0, 128, 512, "dve"), (1, 1, 128, 512, "dve"),
                         (2, 0, 256, 512, "dve"), (2, 1, 256, 512, "dve"),
                         (3, 0, 384, 512, "dve"), (3, 1, 384, 512, "dve")]

                pending = []   # AV chunks of the previous pair
                for a in range(NPAIR):
                    att = att2[a % 2]
                    qa, ka = qkT[2 * a], qkT[2 * a + 1]
                    mine = av_chunks(a) if a == NPAIR - 1 else None
                    for pi, (tk, r, lo, hi, eng) in enumerate(ORDER):
                        qk_piece(att, qa, ka, tk, r, lo, hi, eng)
                        if pending:
                            pending.pop(0)()
                        if mine is not None and pi >= 19:
                            # final pair: bank-high groups slot in behind
                            # the pieces they depend on
                            mine.pop(0)()
                    while pending:
                        pending.pop(0)()
                    if mine is not None:
                        for c in mine:
                            c()
                    else:
                        pending = av_chunks(a)

    nc.compile()
    return nc


def _prep_host(x, W_attn, b_attn):
    s = 1.0 / np.sqrt(np.float32(HD))
    W = np.asarray(W_attn, dtype=np.float32).copy()
    b = np.asarray(b_attn, dtype=np.float32).copy()
    W[:C] *= s
    b[:C] *= s
    # interleave q/k head pairs: [q-pair0, k-pair0, q-pair1, k-pair1, ...], v natural
    rows = []
    for a in range(NPAIR):
        rows.extend(range(128 * a, 128 * (a + 1)))          # q heads 2a, 2a+1
        rows.extend(range(C + 128 * a, C + 128 * (a + 1)))  # k heads 2a, 2a+1
    rows.extend(range(2 * C, 3 * C))                        # v natural
    W_perm = W[rows]
    b_perm = b[rows]

    e4 = ml_dtypes.float8_e4m3

    def pack(mat):
        # (C, N) -> partition-major (128, KT, N): each partition's six
        # contraction k-tiles contiguous, k-pair-major
        Cr, N = mat.shape
        return np.ascontiguousarray(
            mat.reshape(KT, 128, N).transpose(1, 0, 2))

    def split8(mat):
        hi = mat.astype(e4)
        lo = (mat - hi.astype(np.float32)).astype(e4)
        return hi, lo

    wT = np.ascontiguousarray(W_perm.T) * np.float32(SW)     # (C, 3C)
    wqh, wql = split8(pack(wT[:, :2 * C]))
    wvh, wvl = split8(pack(wT[:, 2 * C:]))
    bqk = np.ascontiguousarray(b_perm[:2 * C].reshape(2 * NPAIR, 128).T)  # (128, 12)
    bvb = np.ascontiguousarray(
        np.broadcast_to(b_perm[2 * C:], (128, C))).astype(np.float16)
    tri = (np.arange(128)[None, :] >= np.arange(128)[:, None]).astype(np.float32)
    m1 = np.ones((128, T), dtype=np.float32)
    m1[:, 0:128] = tri             # kept windows always start at the diagonal
    masks = np.ascontiguousarray(
        np.broadcast_to(m1[:, None, :], (128, 2, T)))
    xT = np.asarray(x, dtype=np.float32).transpose(0, 2, 1) * np.float32(SX)  # (B, C, T)
    xhv = np.stack([pack(xT[c]) for c in range(B)])
    xhv, xlv = split8(xhv)
    return xhv, xlv, wqh, wql, wvh, wvl, bqk, bvb, masks


def kernel(x, W_attn, b_attn):
    if "nc" not in _CACHE:
        _CACHE["nc"] = _build()
    nc = _CACHE["nc"]

    (xhv, xlv, wqh, wql, wvh, wvl, bqk, bvb, masks) = _prep_host(x, W_attn, b_attn)
    in_maps = [
        {"xh": xhv[c], "xl": xlv[c], "wqh": wqh, "wql": wql, "wvh": wvh,
         "wvl": wvl, "bqk": bqk, "bvb": bvb, "masks": masks}
        for c in range(B)
    ]
    res = run_bass_kernel_spmd(nc, in_maps, list(range(B)))
    y = np.empty((B, T, C), dtype=np.float32)
    for c in range(B):
        y[c] = res.results[c]["y"].reshape(T, C).astype(np.float32)
    return y


# revision 10
# speedup vs baseline: 1.1352x; 1.0260x over previous
"""Trainium2 Bass kernel for causal masked-ReLU attention (no softmax).

Reference computation (B=8, T=1024, C=768, n_head=12, hd=64):
    qkv = x @ W_attn.T + b_attn
    q, k, v = split(qkv); per-head: att = relu(mask_causal(q k^T / sqrt(hd)))
    y = att @ v, heads re-merged -> (B, T, C)

Sharding: one batch element per NeuronCore (8 cores). Each core computes the
QKV projection and all 12 heads' attention for its batch element.

Layout strategy (per core):
  - Host passes x[b].T (C, T) and W.T (C, 3C) so the contraction dim C lands
    on SBUF partitions with unit-stride DMA (no on-chip transposes).
  - W rows are pre-permuted on host into [q-pair0, k-pair0, q-pair1, ...] so
    q.T / k.T of head h live at the same partition offset (h%2)*64 of their
    M-tiles; q weights/bias pre-scaled by 1/sqrt(hd).
  - QKV projection runs in fp8 (e4m3) DoubleRow perf mode: 256-deep
    contraction per pass at 0.5 cycles/row. Operands split into hi+lo fp8
    digits and three digit products xh*wh + xl*wh + xh*wl accumulate in one
    fp32 PSUM group (dropped xl*wl ~1e-4 relative); the 2^13 operand scale
    comes out at eviction with the bias. PE cost 0.75x of fp16.
  - att is computed transposed (att.T = k @ q.T, layout [T_k, T_q]), fp16
    everywhere, BOTH heads of a pair sharing one [128, 2, 512] double-bank
    PSUM tile and one [128, 2, T] SBUF tile per k-tile: the per-instruction
    PSUM-access penalty on DVE/ACT (~125/185ns) dominates phase 2, and
    pair-merging halves the eviction op count.
  - att tiles are double-buffered ACROSS pairs (sets alternate a%2): pair
    a+1's QK pieces stream through DVE/ACT while pair a's AV still reads
    the other set, so the mask-bound DVE runs and relu-bound ACT runs
    interleave instead of convoying (Pool cannot read PSUM, so evictions
    have only these two engines).
  - Per-pair piece order tk4..tk7, then [512:T) halves of tk0-3, then
    diagonal halves of tk0-3: AV bank-high's dependencies resolve
    mid-pair and bank-low's at the end, so the closing chain after the
    final AV matmul is one [128, 256] copy + one small DMA.
  - The AV product exploits weight-stationary asymmetry: per (q-tile,
    k-tile, head) matmul the STATIONARY operand is the [128, 128] att.T
    block and the MOVING operand is the head's 64 v columns (Ldweights is
    free; cost follows moving columns). Both heads accumulate into one
    [128, 512] PSUM bank (cols = 128*(t%4) + 64*head_parity + d); y
    DMAs out with a 3-d strided AP into natural (T, C) layout.
  - Causal masking at eviction via a [tri(128) | ones] relu-mask; DVE owns
    mask-needing pieces (only PSUM-capable engine with
    scalar_tensor_tensor), ACT the pure-relu halves; tile 4 splits
    [masked 2x128 | relu 2x384] to converge both at ~6.0us/pair, just
    above PE's 5.8us/pair.
  - The input front is split fine (x column halves) and spread across the
    SP HWDGE queue and the Pool SWDGE queue (which bypasses the shared
    HWDGE's ~625ns/DMA): the first window's eviction gates on all three
    contraction pairs, and a single queue starves the PE ~3us.
  - Warmup matmuls on a zeroed scratch tile ramp the PE p-state during the
    initial DMA wait, with more sprinkled into the first windows' stall
    points (idle gaps reset the ~3us ramp).
  - Projection windows: 2 groups per double-bank tile, 3 tiles rotating;
    mid-stream windows borrow the two spare ps_y banks for 8 groups in
    flight. One pool scope spans both phases (pool close = all-engine
    barrier); pair 0's first two k-tiles run unmerged through ps_y so
    phase 2 overlaps the last projection windows.
  - Output is written as y (T, C) in fp16; host upcasts.
"""

import numpy as np

import sys
for _p in ("/opt/trn_rl_repo", "/root/.axon_site", "/root/.axon_site/_ro/trn_rl_repo",
           "/root/.axon_site/_ro/pypackages"):
    if _p not in sys.path:
        sys.path.append(_p)

import ml_dtypes

import concourse.bacc as bacc
import concourse.mybir as mybir
from concourse.alu_op_type import AluOpType
from concourse.tile import TileContext
from concourse.bass_utils import run_bass_kernel_spmd

B, T, C = 8, 1024, 768
NH, HD = 12, 64
C3 = 3 * C            # 2304
KT = C // 128         # 6  contraction tiles of the projection
NP = KT // 2          # 3  contraction pairs (DoubleRow)
TT = T // 128         # 8  tiles of the sequence dim
NPAIR = NH // 2       # 6  head pairs
F32 = mybir.dt.float32
F16 = mybir.dt.float16
F8 = mybir.dt.float8e4
AF = mybir.ActivationFunctionType
DR = mybir.MatmulPerfMode.DoubleRow

SX = 16.0             # x pre-scale (keeps x-lo digits in e4m3 normal range)
SW = 512.0            # W pre-scale
DESCALE = 1.0 / (SX * SW)

# warmup matmul moving-widths (fp16): ramp the PE p-state during the
# initial input-DMA wait so real matmuls start at full clock
WARM = [512] * 4 + [256]
NSPRINKLE = 4         # 256-wide ramp-guard warmups per early stall point

_CACHE = {}


def _build():
    nc = bacc.Bacc("TRN2", target_bir_lowering=False, debug=False, num_devices=8)

    xh = nc.dram_tensor("xh", [128, KT, T], F8, kind="ExternalInput").ap()
    xl = nc.dram_tensor("xl", [128, KT, T], F8, kind="ExternalInput").ap()
    wvh = nc.dram_tensor("wvh", [128, KT, C], F8, kind="ExternalInput").ap()
    wvl = nc.dram_tensor("wvl", [128, KT, C], F8, kind="ExternalInput").ap()
    wqh = nc.dram_tensor("wqh", [128, KT, 2 * C], F8, kind="ExternalInput").ap()
    wql = nc.dram_tensor("wql", [128, KT, 2 * C], F8, kind="ExternalInput").ap()
    bqk = nc.dram_tensor("bqk", [128, 2 * NPAIR], F32, kind="ExternalInput").ap()
    bvb = nc.dram_tensor("bvb", [128, C], F16, kind="ExternalInput").ap()
    # masks = [tri(128) | ones(896)] duplicated along dim1 so pair-merged
    # [128, 2, W] evictions read the same relu-mask for both heads
    masks = nc.dram_tensor("masks", [128, 2, T], F32, kind="ExternalInput").ap()
    # y in natural (T, C) layout, tiled (TT, 128, C) for the 3-d AV DMAs
    y_d = nc.dram_tensor("y", [TT, 128, C], F16, kind="ExternalOutput").ap()

    with TileContext(nc) as tc:
        with (
            tc.tile_pool(name="persist", bufs=1) as pp,
        ):
            masks_sb = pp.tile([128, 2, T], F32, name="masks_sb")
            bqk_sb = pp.tile([128, 2 * NPAIR], F32, name="bqk_sb")
            bvb_sb = pp.tile([128, C], F16, name="bvb_sb")
            qkT = [pp.tile([128, T], F16, name=f"qkT{m}") for m in range(2 * NPAIR)]
            v_sb = [pp.tile([128, C], F16, name=f"v{t}") for t in range(TT)]
            # att.T tiles, fp16, dim1 = head parity; two sets alternating
            # per pair so pair a+1's QK overlaps pair a's AV
            att2 = [[pp.tile([128, 2, T], F16, name=f"att{s}_{t}")
                     for t in range(TT)] for s in range(2)]

            # ---------- Phase 1: QKV projection (fp8 DoubleRow, 3 digit
            # products into one PSUM group) ----------
            from contextlib import ExitStack
            with ExitStack() as stack:
                iop = stack.enter_context(tc.tile_pool(name="io", bufs=1))
                # 6 single-bank tiles for projection windows and phase-2
                # QK pieces (a DEEP ring: eviction backlog depth is what
                # lets DVE/ACT run at ~100%), + 2 single banks for
                # warmup / window-borrow / AV
                ps_proj = stack.enter_context(
                    tc.tile_pool(name="psum_qk", bufs=6, space="PSUM"))
                ps_av = stack.enter_context(
                    tc.tile_pool(name="psum_av", bufs=2, space="PSUM"))
                yop = stack.enter_context(tc.tile_pool(name="yout", bufs=4))
                xh_sb = iop.tile([128, KT, T], F8, name="xh_sb")
                xl_sb = iop.tile([128, KT, T], F8, name="xl_sb")
                wv_h = iop.tile([128, KT, C], F8, name="wv_h")
                wv_l = iop.tile([128, KT, C], F8, name="wv_l")
                wq_h = iop.tile([128, KT, 2 * C], F8, name="wq_h")
                wq_l = iop.tile([128, KT, 2 * C], F8, name="wq_l")

                # PE p-state warmup on a zeroed scratch tile; results are
                # never read
                scratch = iop.tile([128, 512], F16, name="warm_src")
                nc.vector.memset(scratch[:], 0.0)
                warm = ps_av.tile([128, 512], F32, name="ps_warm",
                                  tag="ps_av")
                for w in WARM:
                    nc.tensor.matmul(warm[:, :w], scratch[:, :128],
                                     scratch[:, :w], start=True, stop=True)

                # input DMAs. The first windows' evictions gate on ALL three
                # contraction pairs, so the front ships x in column halves
                # and splits across the SP HWDGE queue and the Pool SWDGE
                # queue (parallel descriptor generators).
                sp, pool = nc.sync, nc.gpsimd
                sp.dma_start(out=wv_h[:, 0:2, :], in_=wvh[:, 0:2, :])
                sp.dma_start(out=xh_sb[:, 0:2, 0:256], in_=xh[:, 0:2, 0:256])
                pool.dma_start(out=xl_sb[:, 0:2, 0:512], in_=xl[:, 0:2, 0:512])
                sp.dma_start(out=wv_l[:, 0:2, :], in_=wvl[:, 0:2, :])
                sp.dma_start(out=xh_sb[:, 0:2, 256:512], in_=xh[:, 0:2, 256:512])
                pool.dma_start(out=xh_sb[:, 2:4, 0:512], in_=xh[:, 2:4, 0:512])
                sp.dma_start(out=wv_h[:, 2:4, :], in_=wvh[:, 2:4, :])
                pool.dma_start(out=xl_sb[:, 2:4, 0:512], in_=xl[:, 2:4, 0:512])
                sp.dma_start(out=wv_l[:, 2:4, :], in_=wvl[:, 2:4, :])
                pool.dma_start(out=xh_sb[:, 4:6, 0:512], in_=xh[:, 4:6, 0:512])
                sp.dma_start(out=wv_h[:, 4:6, :], in_=wvh[:, 4:6, :])
                pool.dma_start(out=xl_sb[:, 4:6, 0:512], in_=xl[:, 4:6, 0:512])
                sp.dma_start(out=wv_l[:, 4:6, :], in_=wvl[:, 4:6, :])
                sp.dma_start(out=bvb_sb[:], in_=bvb[:])
                # x column rests (v tiles 4-7 of the seq dim; windows 2-3)
                for p in range(NP):
                    pr = slice(2 * p, 2 * p + 2)
                    sp.dma_start(out=xh_sb[:, pr, 512:T], in_=xh[:, pr, 512:T])
                    pool.dma_start(out=xl_sb[:, pr, 512:T], in_=xl[:, pr, 512:T])
                # q/k weights; pair 0 in m0-m3 / m4-m11 halves
                pr0 = slice(0, 2)
                sp.dma_start(out=wq_h[:, pr0, :512], in_=wqh[:, pr0, :512])
                pool.dma_start(out=wq_l[:, pr0, :512], in_=wql[:, pr0, :512])
                sp.dma_start(out=wq_h[:, pr0, 512:], in_=wqh[:, pr0, 512:])
                pool.dma_start(out=wq_l[:, pr0, 512:], in_=wql[:, pr0, 512:])
                sp.dma_start(out=bqk_sb[:], in_=bqk[:])
                for p in range(1, NP):
                    prp = slice(2 * p, 2 * p + 2)
                    sp.dma_start(out=wq_h[:, prp, :], in_=wqh[:, prp, :])
                    pool.dma_start(out=wq_l[:, prp, :], in_=wql[:, prp, :])
                sp.dma_start(out=masks_sb[:], in_=masks[:])

                # each group = one [128, 512] PSUM bank lane.
                # ("v", t, n0, width) / ("qk", m, q0, width)
                groups = []
                for t in range(TT):
                    groups.append(("v", t, 0, 512))
                    groups.append(("v", t, 512, 256))
                for m in range(2 * NPAIR):
                    for q0 in (0, 512):
                        groups.append(("qk", m, q0, 512))

                # windows of 4 groups; k-pair-major, digit-product-minor so
                # PE consumption order matches DMA arrival order. Mid-stream
                # windows borrow the two ps_y banks for deeper pipelining.
                nwin = (len(groups) + 3) // 4
                for wi, w0 in enumerate(range(0, len(groups), 4)):
                    window = groups[w0:w0 + 4]
                    borrow = 3 <= wi < nwin - 2
                    tiles = []
                    for gi in range(len(window)):
                        if borrow and gi >= 3:
                            tiles.append(ps_av.tile([128, 512], F32,
                                                    name="ps_b", tag="ps_av"))
                        else:
                            tiles.append(ps_proj.tile([128, 512], F32,
                                                      name="ps_w",
                                                      tag="ps_qk"))
                    nmm = [0] * len(window)
                    total = [9 * (g[3] // 256) for g in window]
                    for p in range(NP):
                        pr = slice(2 * p, 2 * p + 2)
                        for term in range(3):
                            xa = (xh_sb, xl_sb, xh_sb)[term]
                            wva = (wv_h, wv_h, wv_l)[term]
                            wqa = (wq_h, wq_h, wq_l)[term]
                            for gi, (g, ps) in enumerate(zip(window, tiles)):
                                kind, i, o0, wd = g
                                for c0 in range(0, wd, 256):
                                    n = nmm[gi]
                                    nmm[gi] = n + 1
                                    st = n == 0
                                    sp_ = n == total[gi] - 1
                                    if kind == "v":
                                        nc.tensor.matmul(
                                            ps[:, c0:c0 + 256],
                                            xa[:, pr, 128 * i:128 * (i + 1)],
                                            wva[:, pr, o0 + c0:o0 + c0 + 256],
                                            start=st, stop=sp_, perf_mode=DR,
                                        )
                                    else:
                                        nc.tensor.matmul(
                                            ps[:, c0:c0 + 256],
                                            wqa[:, pr, 128 * i:128 * (i + 1)],
                                            xa[:, pr, o0 + c0:o0 + c0 + 256],
                                            start=st, stop=sp_, perf_mode=DR,
                                        )
                            if wi < 2 and term == 2:
                                # ramp-guard warmups at the early stall
                                # points (pair p+1 still in flight)
                                for _ in range(NSPRINKLE):
                                    nc.tensor.matmul(
                                        warm[:, :256], scratch[:, :128],
                                        scratch[:, :256], start=True,
                                        stop=True)
                    for g, ps in zip(window, tiles):
                        kind, i, o0, wd = g
                        if kind == "v":
                            nc.vector.scalar_tensor_tensor(
                                v_sb[i][:, o0:o0 + wd], ps[:, :wd], DESCALE,
                                bvb_sb[:, o0:o0 + wd],
                                AluOpType.mult, AluOpType.add,
                            )
                        elif i % 2 == 0:
                            nc.scalar.activation(
                                qkT[i][:, o0:o0 + wd], ps[:, :wd],
                                AF.Identity, bias=bqk_sb[:, i:i + 1],
                                scale=DESCALE,
                            )
                        else:
                            nc.vector.tensor_scalar(
                                qkT[i][:, o0:o0 + wd], ps[:, :wd],
                                DESCALE, bqk_sb[:, i:i + 1],
                                AluOpType.mult, AluOpType.add,
                            )

            # ---------- Phase 2: attention, pair by pair (same pool
            # scope: no phase barrier) ----------
                def qk_piece(att, qa, ka, tk, r, lo, hi, engine):
                    """One per-head QK piece covering q cols [lo, hi);
                    engine: 'dve' masked relu / 'act' pure relu / 'mix'
                    masked first 128 cols on DVE + relu rest on ACT."""
                    k0 = 128 * tk
                    kh = ka[64 * r:64 * (r + 1), :]
                    qh = qa[64 * r:64 * (r + 1), :]
                    ps = ps_proj.tile([128, 512], F32, name="ps_qk",
                                      tag="ps_qk")
                    nc.tensor.matmul(
                        ps[:, :hi - lo], kh[:, k0:k0 + 128], qh[:, lo:hi],
                        start=True, stop=True,
                    )
                    if engine == "act":
                        nc.scalar.activation(att[tk][:, r, lo:hi],
                                             ps[:, :hi - lo], AF.Relu)
                    elif engine == "dve":
                        nc.vector.scalar_tensor_tensor(
                            att[tk][:, r, lo:hi], ps[:, :hi - lo],
                            0.0, masks_sb[:, 0, :hi - lo],
                            AluOpType.max, AluOpType.mult,
                        )
                    else:  # mix: diag block on DVE, remainder on ACT
                        nc.vector.scalar_tensor_tensor(
                            att[tk][:, r, lo:lo + 128], ps[:, 0:128],
                            0.0, masks_sb[:, 0, :128],
                            AluOpType.max, AluOpType.mult,
                        )
                        nc.scalar.activation(att[tk][:, r, lo + 128:hi],
                                             ps[:, 128:hi - lo], AF.Relu)

                def av_chunks(a, last=False):
                    # AV emission chunks for pair a: one closure per
                    # (bank, q-tile) group plus eviction/DMA closures;
                    # banks live in separate single-bank tiles so each
                    # frees as soon as its own eviction completes
                    att = att2[a % 2]
                    state = {}

                    def group(bk, t):
                        def emit():
                            if bk not in state:
                                state[bk] = ps_av.tile(
                                    [128, 512], F32, name="ps_av",
                                    tag="ps_av")
                            ps2 = state[bk]
                            for r in range(2):
                                h = 2 * a + r
                                col = 128 * (t % 4) + 64 * r
                                for k in range(t + 1):
                                    nc.tensor.matmul(
                                        ps2[:, col:col + 64],
                                        att[k][:, r, 128 * t:128 * (t + 1)],
                                        v_sb[k][:, 64 * h:64 * (h + 1)],
                                        start=(k == 0), stop=(k == t),
                                    )
                        return emit

                    def evict_h():
                        # bank-high: split across both engines, one DMA
                        ps2 = state[1]
                        y_sb = yop.tile([128, 512], F16, name="y_sb",
                                        tag="y_sb")
                        nc.vector.tensor_scalar(
                            y_sb[:, 0:256], ps2[:, 0:256], 0.0, None,
                            AluOpType.add)
                        nc.scalar.copy(y_sb[:, 256:512], ps2[:, 256:512])
                        nc.sync.dma_start(
                            out=y_d[4:8, :, 128 * a:128 * (a + 1)]
                                .transpose([1, 0, 2]),
                            in_=y_sb[:])

                    def evict_l():
                        # bank-low: one whole ACT copy, one DMA
                        ps2 = state[0]
                        y_sb = yop.tile([128, 512], F16, name="y_sb",
                                        tag="y_sb")
                        nc.scalar.copy(y_sb[:], ps2[:])
                        nc.sync.dma_start(
                            out=y_d[0:4, :, 128 * a:128 * (a + 1)]
                                .transpose([1, 0, 2]),
                            in_=y_sb[:])

                    def evict_l_half(half, eng):
                        # final pair: per-half evictions so the closing
                        # chain after the last matmul is small
                        def emit():
                            ps2 = state[0]
                            y_sb = yop.tile([128, 256], F16, name="y_sbq",
                                            tag="y_sbq")
                            hs = slice(256 * half, 256 * (half + 1))
                            if eng == "act":
                                nc.scalar.copy(y_sb[:], ps2[:, hs])
                            else:
                                nc.vector.tensor_scalar(
                                    y_sb[:], ps2[:, hs], 0.0, None,
                                    AluOpType.add)
                            nc.sync.dma_start(
                                out=y_d[2 * half:2 * half + 2, :,
                                        128 * a:128 * (a + 1)]
                                    .transpose([1, 0, 2]),
                                in_=y_sb[:])
                        return emit

                    if last:
                        return ([group(1, t) for t in range(4, 8)]
                                + [evict_h]
                                + [group(0, 0), group(0, 1),
                                   evict_l_half(0, "act"),
                                   group(0, 2), group(0, 3),
                                   evict_l_half(1, "dve")])
                    return ([group(1, t) for t in range(4, 8)] + [evict_h]
                            + [group(0, t) for t in range(4)] + [evict_l])

                # per-pair piece order: tk4 + the [512:T) halves early
                # (bank-high AV deps), diagonal halves of tk0-3 last
                # (bank-low deps), DVE/ACT interleaved where possible
                ORDER = [(4, 0, 512, T, "mix"), (4, 1, 512, T, "mix"),
                         (0, 0, 512, T, "act"), (5, 0, 640, T, "dve"),
                         (0, 1, 512, T, "act"), (5, 1, 640, T, "dve"),
                         (1, 0, 512, T, "act"), (6, 0, 768, T, "dve"),
                         (1, 1, 512, T, "act"), (6, 1, 768, T, "dve"),
                         (2, 0, 512, T, "act"), (7, 0, 896, T, "dve"),
                         (2, 1, 512, T, "act"), (7, 1, 896, T, "dve"),
                         (3, 0, 512, T, "act"), (0, 0, 0, 512, "dve"),
                         (3, 1, 512, T, "act"), (0, 1, 0, 512, "dve"),
                         (1, 0, 128, 512, "dve"), (1, 1, 128, 512, "dve"),
                         (2, 0, 256, 512, "dve"), (2, 1, 256, 512, "dve"),
                         (3, 0, 384, 512, "dve"), (3, 1, 384, 512, "dve")]

                pending = []   # AV chunks of the previous pair
                for a in range(NPAIR):
                    att = att2[a % 2]
                    qa, ka = qkT[2 * a], qkT[2 * a + 1]
                    mine = av_chunks(a, last=True) if a == NPAIR - 1 else None
                    for pi, (tk, r, lo, hi, eng) in enumerate(ORDER):
                        qk_piece(att, qa, ka, tk, r, lo, hi, eng)
                        if pending and pi % 2 == 0:
                            pending.pop(0)()
                        if mine is not None and pi >= 17:
                            # final pair: its bank-high chunks slot in
                            # behind the pieces they depend on
                            mine.pop(0)()
                    while pending:
                        pending.pop(0)()
                    if mine is not None:
                        for c in mine:
                            c()
                    else:
                        pending = av_chunks(a)

    nc.compile()
    return nc


def _prep_host(x, W_attn, b_attn):
    s = 1.0 / np.sqrt(np.float32(HD))
    W = np.asarray(W_attn, dtype=np.float32).copy()
    b = np.asarray(b_attn, dtype=np.float32).copy()
    W[:C] *= s
    b[:C] *= s
    # interleave q/k head pairs: [q-pair0, k-pair0, q-pair1, k-pair1, ...], v natural
    rows = []
    for a in range(NPAIR):
        rows.extend(range(128 * a, 128 * (a + 1)))          # q heads 2a, 2a+1
        rows.extend(range(C + 128 * a, C + 128 * (a + 1)))  # k heads 2a, 2a+1
    rows.extend(range(2 * C, 3 * C))                        # v natural
    W_perm = W[rows]
    b_perm = b[rows]

    e4 = ml_dtypes.float8_e4m3

    def pack(mat):
        # (C, N) -> partition-major (128, KT, N): each partition's six
        # contraction k-tiles contiguous, k-pair-major
        Cr, N = mat.shape
        return np.ascontiguousarray(
            mat.reshape(KT, 128, N).transpose(1, 0, 2))

    def split8(mat):
        hi = mat.astype(e4)
        lo = (mat - hi.astype(np.float32)).astype(e4)
        return hi, lo

    wT = np.ascontiguousarray(W_perm.T) * np.float32(SW)     # (C, 3C)
    wqh, wql = split8(pack(wT[:, :2 * C]))
    wvh, wvl = split8(pack(wT[:, 2 * C:]))
    bqk = np.ascontiguousarray(b_perm[:2 * C].reshape(2 * NPAIR, 128).T)  # (128, 12)
    bvb = np.ascontiguousarray(
        np.broadcast_to(b_perm[2 * C:], (128, C))).astype(np.float16)
    tri = (np.arange(128)[None, :] >= np.arange(128)[:, None]).astype(np.float32)
    m1 = np.ones((128, T), dtype=np.float32)
    m1[:, 0:128] = tri             # kept windows always start at the diagonal
    masks = np.ascontiguousarray(
        np.broadcast_to(m1[:, None, :], (128, 2, T)))
    xT = np.asarray(x, dtype=np.float32).transpose(0, 2, 1) * np.float32(SX)  # (B, C, T)
    xhv = np.stack([pack(xT[c]) for c in range(B)])
    xhv, xlv = split8(xhv)
    return xhv, xlv, wqh, wql, wvh, wvl, bqk, bvb, masks


def kernel(x, W_attn, b_attn):
    if "nc" not in _CACHE:
        _CACHE["nc"] = _build()
    nc = _CACHE["nc"]

    (xhv, xlv, wqh, wql, wvh, wvl, bqk, bvb, masks) = _prep_host(x, W_attn, b_attn)
    in_maps = [
        {"xh": xhv[c], "xl": xlv[c], "wqh": wqh, "wql": wql, "wvh": wvh,
         "wvl": wvl, "bqk": bqk, "bvb": bvb, "masks": masks}
        for c in range(B)
    ]
    res = run_bass_kernel_spmd(nc, in_maps, list(range(B)))
    y = np.empty((B, T, C), dtype=np.float32)
    for c in range(B):
        y[c] = res.results[c]["y"].reshape(T, C).astype(np.float32)
    return y
